# revision 22
# baseline (speedup 1.0000x reference)
"""CensNet Trainium2 kernel — 8-core SPMD Bass/Tile implementation.

Computation (reference semantics):
  gc1: Xh = relu(P @ (X @ W1) + b1)   with P = sym-normalized (A+I) from edge_index
  Zh = relu(Z)
  3x edge layers (p2/W2, p3/W3, p32/W32):
      sv = Xh @ p.T                      [n]
      m2 = T.T @ (T * sv[:,None])        [m,m]  (symmetric)
      A  = (m2 with diag<-1) * adj_e
      Zh = relu(A @ (Zh@W / colmax(A)) + b)     (colmax folded into right operand)
  gc4: se = Zh @ p4.T; X4 = (T*se) @ (T.T @ (P @ (Xh@W4) + b4g)) + b4   (associativity;
       never materializes the [n,n] multiplier)
  gc5: softmax(P @ (relu(X4) @ W5) + b5, axis=1)

Sharding: edge dim m row-sharded 8 ways (750 rows/core) for all m x m work;
node dim n sharded 375 rows/core for P-products. Weights replicated. Collectives:
AllGather (XhT, ZhT pieces), AllReduce (colmax max, G, VT sums).
"""

import sys

for _p in ("/opt/trn_rl_repo", "/root/.axon_site/_ro/trn_rl_repo"):
    if _p not in sys.path:
        sys.path.insert(0, _p)

import numpy as np

import concourse.bass as bass
import concourse.mybir as mybir
import concourse.tile as tile
from concourse import bacc, bass_utils
from concourse.masks import make_identity

F32 = mybir.dt.float32
BF16 = mybir.dt.bfloat16
F8 = mybir.dt.float8e4
PM_DR = mybir.MatmulPerfMode.DoubleRow
AF = mybir.ActivationFunctionType
ALU = mybir.AluOpType

CORES = 8
N, M = 3000, 6000
FV, FE, H1, H2, NCLS = 128, 64, 256, 128, 16
MR = M // CORES   # 750 edge rows per core
MRP = 752         # fp8 LDW needs 4-byte-aligned plane stride
NR = N // CORES   # 375 node rows per core


def _chunks(total, step):
    return [(s, min(step, total - s)) for s in range(0, total, step)]


NCH = _chunks(N, 128)      # 24 contraction chunks over n
NPAD = 3072                # n padded to 12 DoubleRow chunks of 256
NDR = 12
KCH = _chunks(M, 128)      # 47 chunks over m (pass-2 contraction)
KBL = _chunks(M, 512)      # 12 streaming blocks over m
IBL = _chunks(MR, 128)     # 6 row blocks within the core's 750 rows
IFB = _chunks(MR, 375)     # 2 free blocks for pass-2 output
NLB = _chunks(NR, 128)     # 3 local node blocks
VFB = _chunks(N, 512)      # 6 free blocks over n for the VT GEMM
RG = [list(range(CORES))]


def _col_layout(vec, p=128):
    """[L] -> [p, ceil(L/p)] chunk-major (column j holds vec[j*p:(j+1)*p])."""
    L = len(vec)
    ncol = (L + p - 1) // p
    out = np.zeros((p, ncol), np.float32)
    for j in range(ncol):
        seg = vec[j * p:(j + 1) * p]
        out[: len(seg), j] = seg
    return out


def build_program():
    nc = bacc.Bacc("TRN2", target_bir_lowering=False, debug=False,
                   num_devices=CORES)

    dp = lambda name, shape, dt=F32: nc.declare_dram_parameter(name, list(shape), dt, isOutput=False)
    tpad_d = dp("tpad", (NPAD, M), F8)
    tslab_d = dp("tslab", (NPAD, MR), BF16)
    ttslab_d = dp("ttslab", (MR, N), BF16)
    adjslab_d = dp("adjslab", (MR, M))
    ptcol_d = dp("ptcol", (N, NR))
    ptrow_d = dp("ptrow", (NR, N))
    xt_d = dp("xt", (FV, N))
    zt_d = dp("zt", (FE, M))
    kiota_d = dp("kiota", (128, 512))
    cidx_d = dp("cidx", (128, 6))
    vfix_d = dp("vfix", (128, 6))
    w1_d = dp("w1", (FV, H1))
    w2_d = dp("w2", (FE, FE))
    w3_d = dp("w3", (FE, FE))
    w32_d = dp("w32", (FE, FE))
    w4_d = dp("w4", (128, 2, H2))
    w5_d = dp("w5", (H2, NCLS))
    pv2_d = dp("pv2", (128, 2))
    pv3_d = dp("pv3", (128, 2))
    pv32_d = dp("pv32", (128, 2))
    pv4_d = dp("pv4", (FE, 1))
    b1_d = dp("b1", (128, 2))
    b2_d = dp("b2", (FE, 1))
    b3_d = dp("b3", (FE, 1))
    b32_d = dp("b32", (FE, 1))
    b4g_d = dp("b4g", (1, H2))
    b4_d = dp("b4", (H2, 1))
    b5_d = dp("b5", (NCLS, 1))
    out_d = nc.declare_dram_parameter("out", [NR, NCLS], F32, isOutput=True)

    layers = [("2", w2_d, pv2_d, b2_d), ("3", w3_d, pv3_d, b3_d), ("32", w32_d, pv32_d, b32_d)]

    with tile.TileContext(nc) as tc:
        with (
            tc.tile_pool(name="const", bufs=1) as cst,
            tc.tile_pool(name="dram", bufs=1, space="DRAM") as dram,
        ):
            # ------- persistent constants / state -------
            ident = cst.tile([128, 128], F32)
            make_identity(nc, ident[:])
            ones512 = cst.tile([128, 512], F32)
            nc.vector.memset(ones512[:], 1.0)
            kiota = cst.tile([128, 512], F32)
            nc.sync.dma_start(kiota[:], kiota_d[:])
            cidx = cst.tile([128, 6], F32)
            nc.sync.dma_start(cidx[:], cidx_d[:])
            vfix = cst.tile([128, 6], F32)
            nc.sync.dma_start(vfix[:], vfix_d[:])
            w1 = cst.tile([FV, H1], F32)
            nc.sync.dma_start(w1[:], w1_d[:])
            wl_sb = {}
            pv_sb = {}
            bl_sb = {}
            for nm, wd, pvd, bd in layers:
                wl_sb[nm] = cst.tile([FE, FE], F32, tag=f"w{nm}", name=f"w{nm}sb")
                nc.sync.dma_start(wl_sb[nm][:], wd[:])
                pv_sb[nm] = cst.tile([128, 2], F32, tag=f"pv{nm}", name=f"pv{nm}sb")
                nc.sync.dma_start(pv_sb[nm][:], pvd[:])
                bl_sb[nm] = cst.tile([FE, 1], F32, tag=f"b{nm}", name=f"b{nm}sb")
                nc.sync.dma_start(bl_sb[nm][:], bd[:])
            w4 = cst.tile([128, 2, H2], F32)
            nc.sync.dma_start(w4[:], w4_d[:])
            w5 = cst.tile([H2, NCLS], F32)
            nc.sync.dma_start(w5[:], w5_d[:])
            pv4 = cst.tile([FE, 1], F32)
            nc.sync.dma_start(pv4[:], pv4_d[:])
            b1 = cst.tile([128, 2], F32)
            nc.sync.dma_start(b1[:], b1_d[:])
            b4g = cst.tile([1, H2], F32)
            nc.sync.dma_start(b4g[:], b4g_d[:])
            b4 = cst.tile([H2, 1], F32)
            nc.sync.dma_start(b4[:], b4_d[:])
            b5 = cst.tile([NCLS, 1], F32)
            nc.sync.dma_start(b5[:], b5_d[:])

            # T slab (lhsT panel for m2 GEMM + U GEMM): [128, 12, 2, 752]
            tslab = cst.tile([128, NDR, 2, MRP], BF16)
            nc.vector.memset(tslab[:], 0.0)
            for dci in range(NDR):
                for j in range(2):
                    r0 = dci * 256 + j * 128
                    nc.sync.dma_start(tslab[:, dci, j, :MR], tslab_d[r0:r0 + 128, :])

            # Zh (transposed layout), updated per edge layer
            zht = cst.tile([FE, M], F32)
            nc.sync.dma_start(zht[:], zt_d[:])
            nc.scalar.activation(zht[:], zht[:], AF.Relu)

            # per-layer sv columns [128, 24] + local-Xh columns, se, etc.
            svcol = {nm: cst.tile([128, len(NCH)], F32, tag=f"sv{nm}", name=f"sv{nm}sb") for nm, *_ in layers}
            xh_loc = cst.tile([128, 2, NR], F32)     # core's own XhT columns
            xw4 = cst.tile([128, len(NLB), H2], F32)  # XW4 for core's node rows
            secol = cst.tile([128, len(IBL)], F32)
            cmax = cst.tile([128, len(KCH)], F32)
            rmax = cst.tile([128, 6016], F32)
            invc = cst.tile([128, len(KCH)], F32)
            enrelu = cst.tile([FE, MR], F32)          # core's relu'd EnT piece
            u_raw = cst.tile([128, len(IBL), H2], F32)  # T.T @ G (pre-se-scale)

            # DRAM scratch
            at_dram = [dram.tile([768, 6016], BF16, tag=f"at{i}", name=f"atdram{i}") for i in range(2)]
            sv_gin = dram.tile([3, NR], F32)
            sv_gout = dram.tile([CORES, 3, NR], F32)
            cm_in = dram.tile([128, len(KCH)], F32)
            cm_out = dram.tile([128, len(KCH)], F32)
            zg_in = dram.tile([FE, MR], F32)
            zg_out = dram.tile([CORES, FE, MR], F32)
            gd_in = dram.tile([N, H2], F32)
            gd_out = dram.tile([N, H2], F32)
            vt_in = dram.tile([H2, N], F32)
            vt_out = dram.tile([H2, N], F32)

            # ================= gc1 =================
            with (
                tc.tile_pool(name="g1", bufs=1) as g1,
                tc.tile_pool(name="g1s", bufs=3) as g1s,
                tc.tile_pool(name="g1p", bufs=2, space="PSUM") as g1p,
                tc.tile_pool(name="g1px", bufs=2, space="PSUM") as g1px,
            ):
                xt_sb = g1.tile([FV, N], F32)
                nc.sync.dma_start(xt_sb[:], xt_d[:])
                xw1 = g1.tile([128, len(NCH), H1], F32)
                for ci, (ns, nsz) in enumerate(NCH):
                    ps = g1p.tile([128, H1], F32)
                    nc.tensor.matmul(ps[:nsz, :], (xt_sb[:, ns:ns + nsz]), (w1[:]),
                                     start=True, stop=True)
                    nc.scalar.copy(xw1[:nsz, ci, :], ps[:nsz, :])
                psx = [g1px.tile([128, NR], F32, tag=f"psx{hb}", name=f"psx{hb}t") for hb in range(2)]
                for ci, (ns, nsz) in enumerate(NCH):
                    ptc = g1s.tile([128, NR], F32, tag="ptc")
                    nc.sync.dma_start(ptc[:nsz, :], ptcol_d[ns:ns + nsz, :])
                    for hb in range(2):
                        nc.tensor.matmul(
                            psx[hb][:, :],
                            (xw1[:nsz, ci, hb * 128:(hb + 1) * 128]),
                            (ptc[:nsz, :]),
                            start=(ci == 0), stop=(ci == len(NCH) - 1))
                for hb in range(2):
                    nc.scalar.activation(xh_loc[:, hb, :], psx[hb][:, :], AF.Relu,
                                         bias=b1[:, hb:hb + 1])

            # ============ prep: local sv pieces -> tiny AllGather; XW4 local ============
            with (
                tc.tile_pool(name="prep", bufs=1) as pr,
                tc.tile_pool(name="prp", bufs=2, space="PSUM") as prp,
            ):
                svp = pr.tile([128, 3, len(NLB)], F32)
                for li, (nm, _, _, _) in enumerate(layers):
                    for j, (nl, nlsz) in enumerate(NLB):
                        ps = prp.tile([128, 1], F32, tag="psv")
                        for hb in range(2):
                            nc.tensor.matmul(ps[:nlsz, :], xh_loc[:, hb, nl:nl + nlsz],
                                             pv_sb[nm][:, hb:hb + 1],
                                             start=(hb == 0), stop=(hb == 1))
                        nc.scalar.copy(svp[:nlsz, li, j:j + 1], ps[:nlsz, :])
                        nc.sync.dma_start(sv_gin[li, nl:nl + nlsz],
                                          svp[:nlsz, li, j:j + 1])
                nc.gpsimd.collective_compute(
                    "AllGather", ALU.bypass, replica_groups=RG,
                    ins=[sv_gin[:].opt()], outs=[sv_gout[:].opt()])
                # scatter gathered sv values into chunk-major [128, 24] columns
                for li, (nm, _, _, _) in enumerate(layers):
                    for ci, (ns, nsz) in enumerate(NCH):
                        lo = ns
                        while lo < ns + nsz:
                            r = lo // NR
                            take = min((r + 1) * NR, ns + nsz) - lo
                            nc.sync.dma_start(
                                svcol[nm][lo - ns:lo - ns + take, ci:ci + 1],
                                sv_gout[r, li, lo - r * NR:lo - r * NR + take]
                                .unsqueeze(-1))
                            lo += take
                for j, (nl, nlsz) in enumerate(NLB):
                    ps = prp.tile([128, H2], F32, tag="psw4")
                    for hb in range(2):
                        nc.tensor.matmul(ps[:nlsz, :], xh_loc[:, hb, nl:nl + nlsz],
                                         w4[:, hb, :], start=(hb == 0), stop=(hb == 1))
                    nc.scalar.copy(xw4[:nlsz, j, :], ps[:nlsz, :])

            # ===== gc4 early: G = P @ XW4 + b4g and U = T.T @ G (independent of Zh) =====
            with (
                tc.tile_pool(name="g4e", bufs=1) as g4e,
                tc.tile_pool(name="g4es", bufs=3) as g4es,
                tc.tile_pool(name="g4ep", bufs=2, space="PSUM") as g4ep,
                tc.tile_pool(name="g4ep1", bufs=1, space="PSUM") as g4ep1,
            ):
                for ci, (ns, nsz) in enumerate(NCH):
                    ps = g4ep.tile([128, H2], F32, tag="psg")
                    for j, (nl, nlsz) in enumerate(NLB):
                        ptr = g4es.tile([128, 128], F32, tag="ptr")
                        nc.sync.dma_start(ptr[:nlsz, :nsz], ptrow_d[nl:nl + nlsz, ns:ns + nsz])
                        nc.tensor.matmul(ps[:nsz, :], ptr[:nlsz, :nsz], xw4[:nlsz, j, :],
                                         start=(j == 0), stop=(j == len(NLB) - 1))
                    gst = g4es.tile([128, H2], F32, tag="gst")
                    nc.scalar.copy(gst[:nsz, :], ps[:nsz, :])
                    nc.sync.dma_start(gd_in[ns:ns + nsz, :], gst[:nsz, :])
                nc.gpsimd.collective_compute(
                    "AllReduce", ALU.add, replica_groups=RG,
                    ins=[gd_in[:].opt()], outs=[gd_out[:].opt()])
                ones1 = g4e.tile([1, 128], F32)
                nc.vector.memset(ones1[:], 1.0)
                psb = g4ep1.tile([128, H2], F32, tag="small")
                nc.tensor.matmul(psb[:, :], ones1[:, :], b4g[:, :], start=True, stop=True)
                b4gb = g4e.tile([128, H2], F32)
                nc.scalar.copy(b4gb[:], psb[:, :])
                g_f32 = g4e.tile([128, len(NCH), H2], F32)
                g_sb = g4e.tile([128, len(NCH), H2], BF16)
                for ci, (ns, nsz) in enumerate(NCH):
                    nc.sync.dma_start(g_f32[:nsz, ci, :], gd_out[ns:ns + nsz, :])
                    nc.vector.tensor_add(g_sb[:nsz, ci, :], g_f32[:nsz, ci, :],
                                         b4gb[:nsz, :])
                for kb, (k0, ksz) in enumerate(IBL):
                    ps = g4ep.tile([128, H2], F32, tag="psu")
                    for ci, (ns, nsz) in enumerate(NCH):
                        nc.tensor.matmul(ps[:ksz, :],
                                         tslab[:nsz, ci // 2, ci % 2, k0:k0 + ksz],
                                         g_sb[:nsz, ci, :],
                                         start=(ci == 0), stop=(ci == len(NCH) - 1))
                    nc.scalar.copy(u_raw[:ksz, kb, :], ps[:ksz, :])

            # ================= edge layers (software-pipelined emission) =================
            with (
                tc.tile_pool(name="ts", bufs=20) as tsp,
                tc.tile_pool(name="aux", bufs=3) as auxp,
                tc.tile_pool(name="abuf", bufs=3) as abufp,
                tc.tile_pool(name="ats", bufs=3) as atsp,
                tc.tile_pool(name="fix", bufs=2) as fixp,
                tc.tile_pool(name="hew", bufs=1) as hewp,
                tc.tile_pool(name="tsvp", bufs=2) as tsvp,
                tc.tile_pool(name="m2p", bufs=3, space="PSUM") as m2p,
                tc.tile_pool(name="tpp", bufs=1, space="PSUM") as tpp,
                tc.tile_pool(name="hwp", bufs=2, space="PSUM") as hwp,
                tc.tile_pool(name="enp", bufs=1, space="PSUM") as enp,
            ):
                vfixb = hewp.tile([128, len(IBL), 512], F32, tag="vfixb")
                for ib in range(len(IBL)):
                    nc.vector.tensor_scalar(vfixb[:, ib, :], ones512[:],
                                            vfix[:, ib:ib + 1], None, op0=ALU.mult)
                tsv_t = {}
                hew_t = {}

                def emit_tsv(li):
                    nm = layers[li][0]
                    t = tsvp.tile([128, NDR, 2, MRP], F8, tag="tsv", name=f"tsv{li}")
                    tsv_t[li] = t
                    for ci in range(len(NCH)):
                        nc.scalar.activation(t[:, ci // 2, ci % 2, :],
                                             tslab[:, ci // 2, ci % 2, :], AF.Copy,
                                             scale=svcol[nm][:, ci:ci + 1])
                    nc.vector.memset(rmax[:], -3.0e38)

                def emit_pass1_kb(li, kb):
                    atd = at_dram[li % 2]
                    tsv = tsv_t[li]
                    k0, kbs = KBL[kb]
                    ts_tiles = []
                    for dci in range(NDR):
                        tst = tsp.tile([128, 2, 512], F8, tag="ts", name=f"ts{li}_{kb}_{dci}")
                        nc.sync.dma_start(
                            tst[:, :, :kbs],
                            tpad_d[dci * 256:(dci + 1) * 256, k0:k0 + kbs]
                            .rearrange("(two p) k -> p two k", p=128))
                        ts_tiles.append(tst)
                    for ib, (i0, ibs) in enumerate(IBL):
                        ibp = (ibs + 3) // 4 * 4
                        pm = m2p.tile([128, 512], F32, tag="pm", name=f"pm{li}_{kb}_{ib}")
                        for dci in range(NDR):
                            nc.tensor.matmul(
                                pm[:ibp, :kbs], tsv[:, dci, :, i0:i0 + ibp],
                                ts_tiles[dci][:, :, :kbs],
                                start=(dci == 0), stop=(dci == NDR - 1),
                                perf_mode=PM_DR)
                        adj = auxp.tile([128, 512], F32, tag="aux", name=f"adj{li}_{kb}_{ib}")
                        nc.sync.dma_start(adj[:ibs, :kbs],
                                          adjslab_d[i0:i0 + ibs, k0:k0 + kbs])
                        asb = abufp.tile([128, 512], F32, tag="a", name=f"a{li}_{kb}_{ib}")
                        nc.vector.tensor_mul(asb[:ibs, :kbs], pm[:ibs, :kbs],
                                             adj[:ibs, :kbs])
                        ck = fixp.tile([128, 1], F32, tag="ck", name=f"ck{li}_{kb}_{ib}")
                        nc.vector.tensor_scalar(ck[:ibs, :], cidx[:ibs, ib:ib + 1],
                                                float(-k0), None, op0=ALU.add)
                        mk = fixp.tile([128, 512], mybir.dt.uint8, tag="mk",
                                       name=f"mk{li}_{kb}_{ib}")
                        nc.vector.tensor_scalar(mk[:ibs, :kbs], kiota[:ibs, :kbs],
                                                ck[:ibs, :], None, op0=ALU.is_equal)
                        nc.vector.copy_predicated(asb[:ibs, :kbs], mk[:ibs, :kbs],
                                                  vfixb[:ibs, ib, :kbs])
                        nc.vector.tensor_max(rmax[:ibs, k0:k0 + kbs],
                                             rmax[:ibs, k0:k0 + kbs], asb[:ibs, :kbs])
                        ats = atsp.tile([128, 512], BF16, tag="ats", name=f"ats{li}_{kb}_{ib}")
                        nc.scalar.copy(ats[:ibs, :kbs], asb[:ibs, :kbs])
                        nc.sync.dma_start(atd[i0:i0 + ibs, k0:k0 + kbs], ats[:ibs, :kbs])

                def emit_cmax_ar(li):
                    for kc, (k0, kcs) in enumerate(KCH):
                        tp = tpp.tile([128, 128], F32, tag="tp", name=f"tpr{li}_{kc}")
                        nc.tensor.transpose(tp[:kcs, :], rmax[:, k0:k0 + kcs], ident[:, :])
                        nc.vector.reduce_max(cmax[:kcs, kc:kc + 1], tp[:kcs, :],
                                             axis=mybir.AxisListType.X)
                    nc.sync.dma_start(cm_in[:], cmax[:])
                    nc.gpsimd.collective_compute(
                        "AllReduce", ALU.max, replica_groups=RG,
                        ins=[cm_in[:].opt()], outs=[cm_out[:].opt()])

                def emit_hew(li):
                    nm = layers[li][0]
                    hw = hewp.tile([128, len(KCH), FE], BF16, tag="hew", name=f"hew{li}")
                    hew_t[li] = hw
                    for kc, (k0, kcs) in enumerate(KCH):
                        ps = hwp.tile([128, FE], F32, tag="pshew", name=f"ph{li}_{kc}")
                        nc.tensor.matmul(ps[:kcs, :], zht[:, k0:k0 + kcs], wl_sb[nm][:],
                                         start=True, stop=True)
                        nc.scalar.copy(hw[:kcs, kc, :], ps[:kcs, :])

                def emit_scale_hew(li):
                    hw = hew_t[li]
                    nc.sync.dma_start(cmax[:], cm_out[:])
                    nc.vector.reciprocal(invc[:], cmax[:])
                    for kc, (k0, kcs) in enumerate(KCH):
                        nc.vector.tensor_scalar_mul(hw[:kcs, kc, :], hw[:kcs, kc, :],
                                                    invc[:kcs, kc:kc + 1])

                def emit_pass2(li, last):
                    nm, _, _, bd = layers[li]
                    atd = at_dram[li % 2]
                    hw = hew_t[li]
                    for fb, f0 in enumerate((0, 384)):
                        fvalid = 384 if fb == 0 else MR - 384
                        pe = enp.tile([FE, 384], F32, tag="pe", name=f"pe{li}_{fb}")
                        for kc, (k0, kcs) in enumerate(KCH):
                            att = auxp.tile([128, 384], BF16, tag="attb",
                                            name=f"att{li}_{fb}_{kc}")
                            nc.sync.dma_start_transpose(
                                att[:, :], atd[f0:f0 + 384, kc * 128:(kc + 1) * 128])
                            nc.tensor.matmul(pe[:, :], hw[:kcs, kc, :], att[:kcs, :],
                                             start=(kc == 0), stop=(kc == len(KCH) - 1))
                        nc.scalar.activation(enrelu[:, f0:f0 + fvalid], pe[:, :fvalid],
                                             AF.Relu, bias=bl_sb[nm][:])
                    if not last:
                        nc.sync.dma_start(zg_in[:], enrelu[:])
                        nc.gpsimd.collective_compute(
                            "AllGather", ALU.bypass, replica_groups=RG,
                            ins=[zg_in[:].opt()], outs=[zg_out[:].opt()])
                        for r in range(CORES):
                            nc.sync.dma_start(zht[:, r * MR:(r + 1) * MR], zg_out[r, :, :])

                # pipelined emission: next layer's pass1 covers this layer's colmax AR
                emit_tsv(0)
                for kb in range(len(KBL)):
                    emit_pass1_kb(0, kb)
                emit_cmax_ar(0)
                emit_hew(0)
                emit_tsv(1)
                for kb in range(3):
                    emit_pass1_kb(1, kb)
                emit_scale_hew(0)
                emit_pass2(0, last=False)
                for kb in range(3, len(KBL)):
                    emit_pass1_kb(1, kb)
                emit_cmax_ar(1)
                emit_hew(1)
                emit_tsv(2)
                for kb in range(3):
                    emit_pass1_kb(2, kb)
                emit_scale_hew(1)
                emit_pass2(1, last=False)
                for kb in range(3, len(KBL)):
                    emit_pass1_kb(2, kb)
                emit_cmax_ar(2)
                emit_hew(2)
                emit_scale_hew(2)
                emit_pass2(2, last=True)

            # ================= gc4 =================
            with (
                tc.tile_pool(name="g4", bufs=1) as g4,
                tc.tile_pool(name="g4s", bufs=3) as g4s,
                tc.tile_pool(name="g4p", bufs=2, space="PSUM") as g4p,
                tc.tile_pool(name="g4p1", bufs=1, space="PSUM") as g4p1,
            ):
                # se from the core's own relu'd Zh piece
                for kb, (k0, ksz) in enumerate(IBL):
                    ps = g4p1.tile([128, 1], F32, tag="small")
                    nc.tensor.matmul(ps[:ksz, :], enrelu[:, k0:k0 + ksz], pv4[:],
                                     start=True, stop=True)
                    nc.scalar.copy(secol[:ksz, kb:kb + 1], ps[:ksz, :])
                # scale precomputed U by se
                u_sb = g4.tile([128, len(IBL), H2], BF16)
                for kb, (k0, ksz) in enumerate(IBL):
                    nc.vector.tensor_scalar_mul(u_sb[:ksz, kb, :], u_raw[:ksz, kb, :],
                                                secol[:ksz, kb:kb + 1])
                # VT partial = (U*se).T-accum over local edge rows x TT slab
                for vb, (v0, vsz) in enumerate(VFB):
                    ps = g4p.tile([H2, 512], F32, tag="psvt")
                    for kb, (k0, ksz) in enumerate(IBL):
                        ttt = g4s.tile([128, 512], BF16, tag="ttt")
                        nc.sync.dma_start(ttt[:ksz, :vsz], ttslab_d[k0:k0 + ksz, v0:v0 + vsz])
                        nc.tensor.matmul(ps[:, :vsz], (u_sb[:ksz, kb, :]),
                                         (ttt[:ksz, :vsz]),
                                         start=(kb == 0), stop=(kb == len(IBL) - 1))
                    vst = g4s.tile([H2, 512], F32, tag="vst")
                    nc.scalar.copy(vst[:, :vsz], ps[:, :vsz])
                    nc.sync.dma_start(vt_in[:, v0:v0 + vsz], vst[:, :vsz])
                nc.gpsimd.collective_compute(
                    "AllReduce", ALU.add, replica_groups=RG,
                    ins=[vt_in[:].opt()], outs=[vt_out[:].opt()])

            # ================= gc5 + softmax =================
            with (
                tc.tile_pool(name="g5", bufs=1) as g5,
                tc.tile_pool(name="g5s", bufs=3) as g5s,
                tc.tile_pool(name="g5p", bufs=2, space="PSUM") as g5p,
                tc.tile_pool(name="g5pt", bufs=1, space="PSUM") as g5pt,
            ):
                xh5t = g5.tile([H2, N], F32)
                nc.sync.dma_start(xh5t[:], vt_out[:])
                nc.scalar.activation(xh5t[:], xh5t[:], AF.Relu, bias=b4[:])
                xw5 = g5.tile([128, len(NCH), NCLS], F32)
                for ci, (ns, nsz) in enumerate(NCH):
                    ps = g5p.tile([128, NCLS], F32, tag="psw5")
                    nc.tensor.matmul(ps[:nsz, :], xh5t[:, ns:ns + nsz], w5[:],
                                     start=True, stop=True)
                    nc.scalar.copy(xw5[:nsz, ci, :], ps[:nsz, :])
                pst = g5pt.tile([NCLS, 375], F32)
                for ci, (ns, nsz) in enumerate(NCH):
                    ptc = g5s.tile([128, NR], F32, tag="ptc5")
                    nc.sync.dma_start(ptc[:nsz, :], ptcol_d[ns:ns + nsz, :])
                    nc.tensor.matmul(pst[:, :], (xw5[:nsz, ci, :]), (ptc[:nsz, :]),
                                     start=(ci == 0), stop=(ci == len(NCH) - 1))
                st_sb = g5.tile([NCLS, NR], F32)
                nc.vector.tensor_scalar_add(st_sb[:], pst[:, :], b5[:])
                outt = g5.tile([128, len(NLB), NCLS], F32)
                ptp = g5pt.tile([128, len(NLB), NCLS], F32)
                for j, (t0, tsz) in enumerate(NLB):
                    nc.tensor.transpose(ptp[:tsz, j, :], st_sb[:, t0:t0 + tsz],
                                        ident[:NCLS, :NCLS])
                    red = g5s.tile([128, 1], F32, tag="red5")
                    nc.vector.reduce_max(red[:tsz, :], ptp[:tsz, j, :],
                                         axis=mybir.AxisListType.X)
                    nc.vector.tensor_scalar_mul(red[:tsz, :], red[:tsz, :], -1.0)
                    nc.scalar.activation(outt[:tsz, j, :], ptp[:tsz, j, :], AF.Exp,
                                         bias=red[:tsz, :])
                    ssum = g5s.tile([128, 1], F32, tag="ssum5")
                    nc.vector.reduce_sum(ssum[:tsz, :], outt[:tsz, j, :],
                                         axis=mybir.AxisListType.X)
                    nc.vector.reciprocal(ssum[:tsz, :], ssum[:tsz, :])
                    nc.vector.tensor_scalar_mul(outt[:tsz, j, :], outt[:tsz, j, :],
                                                ssum[:tsz, :])
                    nc.sync.dma_start(out_d[t0:t0 + tsz, :], outt[:tsz, j, :])

    nc.finalize()
    return nc


def prepare_inputs(inputs):
    f = lambda x: np.ascontiguousarray(np.asarray(x), dtype=np.float32)
    X, Z, adj_e, T = f(inputs["X"]), f(inputs["Z"]), f(inputs["adj_e"]), f(inputs["T"])
    ei = np.asarray(inputs["edge_index"])
    W1, b1 = f(inputs["W1"]), f(inputs["b1"])
    p2, W2, b2 = f(inputs["p2"]), f(inputs["W2"]), f(inputs["b2"])
    p3, W3, b3 = f(inputs["p3"]), f(inputs["W3"]), f(inputs["b3"])
    p32, W32, b32 = f(inputs["p32"]), f(inputs["W32"]), f(inputs["b32"])
    p4, W4 = f(inputs["p4"]), f(inputs["W4"])
    b4g, b4, W5, b5 = f(inputs["b4g"]), f(inputs["b4"]), f(inputs["W5"]), f(inputs["b5"])

    # dense PT = P.T where P is the symmetric-normalized (A+I) propagation matrix
    src = ei[0].astype(np.int64)
    dst = ei[1].astype(np.int64)
    loop = np.arange(N, dtype=np.int64)
    s = np.concatenate([src, loop])
    d = np.concatenate([dst, loop])
    deg = np.zeros(N, np.float32)
    np.add.at(deg, d, np.float32(1.0))
    dinv = np.where(deg > 0, 1.0 / np.sqrt(deg), 0.0).astype(np.float32)
    norm = dinv[s] * dinv[d]
    PT = np.zeros((N, N), np.float32)
    np.add.at(PT, (s, d), norm)

    import ml_dtypes
    bf16 = ml_dtypes.bfloat16
    fp8 = ml_dtypes.float8_e4m3
    Tpad = np.zeros((3072, M), np.float32)
    Tpad[:N] = T
    kiota = np.tile(np.arange(512, dtype=np.float32), (128, 1))
    base = dict(
        tpad=np.ascontiguousarray(Tpad.astype(fp8)),
        xt=np.ascontiguousarray(X.T), zt=np.ascontiguousarray(Z.T),
        kiota=kiota, w1=W1, w2=W2, w3=W3, w32=W32,
        w4=np.ascontiguousarray(np.transpose(W4.reshape(2, 128, H2), (1, 0, 2))),
        w5=W5,
        pv2=np.ascontiguousarray(p2[0].reshape(2, 128).T),
        pv3=np.ascontiguousarray(p3[0].reshape(2, 128).T),
        pv32=np.ascontiguousarray(p32[0].reshape(2, 128).T),
        pv4=np.ascontiguousarray(p4[0][:, None]),
        b1=np.ascontiguousarray(b1.reshape(2, 128).T),
        b2=b2[:, None], b3=b3[:, None], b32=b32[:, None],
        b4g=b4g[None, :], b4=b4[:, None], b5=b5[:, None],
    )
    in_maps = []
    for c in range(CORES):
        e0, n0 = c * MR, c * NR
        m = dict(base)
        m["tslab"] = np.ascontiguousarray(Tpad[:, e0:e0 + MR].astype(bf16))
        m["ttslab"] = np.ascontiguousarray(T[:, e0:e0 + MR].T.astype(bf16))
        m["adjslab"] = np.ascontiguousarray(adj_e[e0:e0 + MR, :])
        m["ptcol"] = np.ascontiguousarray(PT[:, n0:n0 + NR])
        m["ptrow"] = np.ascontiguousarray(PT[n0:n0 + NR, :])
        m["cidx"] = _col_layout(np.arange(e0, e0 + MR, dtype=np.float32), 128)
        m["cidx"][110:, 5] = -1.0  # pad slots beyond row 750 must never match
        m["vfix"] = _col_layout(np.diagonal(adj_e[e0:e0 + MR, e0:e0 + MR]).astype(np.float32), 128)
        in_maps.append({k: (np.ascontiguousarray(v) if v.dtype in (bf16, fp8)
                            else np.ascontiguousarray(v, dtype=np.float32))
                        for k, v in m.items()})
    return in_maps


_CACHE = {}


def kernel(**inputs):
    in_maps = prepare_inputs(inputs)
    if "nc" not in _CACHE:
        _CACHE["nc"] = build_program()
    res = bass_utils.run_bass_kernel_spmd(_CACHE["nc"], in_maps, list(range(CORES)))
    out = np.concatenate([res.results[c]["out"] for c in range(CORES)], axis=0)
    return out.astype(np.float32)


if __name__ == "__main__":
    import reference
    ins = reference.setup_inputs()
    ins = {k: np.asarray(v) for k, v in ins.items()}
    got = kernel(**ins)
    print("kernel output", got.shape, got.dtype)


# revision 25
# speedup vs baseline: 1.2860x; 1.2860x over previous
"""CensNet Trainium2 kernel — 8-core SPMD Bass/Tile implementation.

Computation (reference semantics):
  gc1: Xh = relu(P @ (X @ W1) + b1)   with P = sym-normalized (A+I) from edge_index
  Zh = relu(Z)
  3x edge layers (p2/W2, p3/W3, p32/W32):
      sv = Xh @ p.T                      [n]
      m2 = T.T @ (T * sv[:,None])        [m,m]  (symmetric)
      A  = (m2 with diag<-1) * adj_e
      Zh = relu(A @ (Zh@W / colmax(A)) + b)     (colmax folded into right operand)
  gc4: se = Zh @ p4.T; X4 = (T*se) @ (T.T @ (P @ (Xh@W4) + b4g)) + b4   (associativity;
       never materializes the [n,n] multiplier)
  gc5: softmax(P @ (relu(X4) @ W5) + b5, axis=1)

Sharding: edge dim m row-sharded 8 ways (750 rows/core) for all m x m work;
node dim n sharded 375 rows/core for P-products. Weights replicated. Collectives:
AllGather (XhT, ZhT pieces), AllReduce (colmax max, G, VT sums).
"""

import sys

for _p in ("/opt/trn_rl_repo", "/root/.axon_site/_ro/trn_rl_repo"):
    if _p not in sys.path:
        sys.path.insert(0, _p)

import numpy as np

import concourse.bass as bass
import concourse.mybir as mybir
import concourse.tile as tile
from concourse import bacc, bass_utils
from concourse.masks import make_identity

F32 = mybir.dt.float32
BF16 = mybir.dt.bfloat16
F8 = mybir.dt.float8e4
PM_DR = mybir.MatmulPerfMode.DoubleRow
AF = mybir.ActivationFunctionType
ALU = mybir.AluOpType

CORES = 8
N, M = 3000, 6000
FV, FE, H1, H2, NCLS = 128, 64, 256, 128, 16
MR = M // CORES   # 750 edge rows per core
MRP = 752         # fp8 LDW needs 4-byte-aligned plane stride
NR = N // CORES   # 375 node rows per core


def _chunks(total, step):
    return [(s, min(step, total - s)) for s in range(0, total, step)]


NCH = _chunks(N, 128)      # 24 contraction chunks over n
NPAD = 3072                # n padded to 12 DoubleRow chunks of 256
NDR = 12
KCH = _chunks(M, 128)      # 47 chunks over m (pass-2 contraction)
KBL = _chunks(M, 512)      # 12 streaming blocks over m
IBL = _chunks(MR, 128)     # 6 row blocks within the core's 750 rows
IFB = _chunks(MR, 375)     # 2 free blocks for pass-2 output
NLB = _chunks(NR, 128)     # 3 local node blocks
VFB = _chunks(N, 512)      # 6 free blocks over n for the VT GEMM
RG = [list(range(CORES))]


def _col_layout(vec, p=128):
    """[L] -> [p, ceil(L/p)] chunk-major (column j holds vec[j*p:(j+1)*p])."""
    L = len(vec)
    ncol = (L + p - 1) // p
    out = np.zeros((p, ncol), np.float32)
    for j in range(ncol):
        seg = vec[j * p:(j + 1) * p]
        out[: len(seg), j] = seg
    return out


def build_program():
    nc = bacc.Bacc("TRN2", target_bir_lowering=False, debug=False,
                   num_devices=CORES)

    dp = lambda name, shape, dt=F32: nc.declare_dram_parameter(name, list(shape), dt, isOutput=False)
    tpad_d = dp("tpad", (NPAD, M), F8)
    tslab_d = dp("tslab", (NPAD, MR), BF16)
    ttslab_d = dp("ttslab", (MR, N), BF16)
    adjslab_d = dp("adjslab", (MR, M))
    ptcol_d = dp("ptcol", (N, NR))
    ptrow_d = dp("ptrow", (NR, N))
    xt_d = dp("xt", (FV, N))
    zt_d = dp("zt", (FE, M))
    kiota_d = dp("kiota", (128, 512))
    cidx_d = dp("cidx", (128, 6))
    vfix_d = dp("vfix", (128, 6))
    w1_d = dp("w1", (FV, H1))
    w2_d = dp("w2", (FE, FE))
    w3_d = dp("w3", (FE, FE))
    w32_d = dp("w32", (FE, FE))
    w4_d = dp("w4", (128, 2, H2))
    w5_d = dp("w5", (H2, NCLS))
    pv2_d = dp("pv2", (128, 2))
    pv3_d = dp("pv3", (128, 2))
    pv32_d = dp("pv32", (128, 2))
    pv4_d = dp("pv4", (FE, 1))
    b1_d = dp("b1", (128, 2))
    b2_d = dp("b2", (FE, 1))
    b3_d = dp("b3", (FE, 1))
    b32_d = dp("b32", (FE, 1))
    b4g_d = dp("b4g", (1, H2))
    b4_d = dp("b4", (H2, 1))
    b5_d = dp("b5", (NCLS, 1))
    out_d = nc.declare_dram_parameter("out", [NR, NCLS], F32, isOutput=True)

    layers = [("2", w2_d, pv2_d, b2_d), ("3", w3_d, pv3_d, b3_d), ("32", w32_d, pv32_d, b32_d)]

    with tile.TileContext(nc) as tc:
        with (
            tc.tile_pool(name="const", bufs=1) as cst,
            tc.tile_pool(name="dram", bufs=1, space="DRAM") as dram,
        ):
            # ------- persistent constants / state -------
            ident = cst.tile([128, 128], F32)
            make_identity(nc, ident[:])
            ones512 = cst.tile([128, 512], F32)
            nc.vector.memset(ones512[:], 1.0)
            kiota = cst.tile([128, 512], F32)
            nc.sync.dma_start(kiota[:], kiota_d[:])
            cidx = cst.tile([128, 6], F32)
            nc.sync.dma_start(cidx[:], cidx_d[:])
            vfix = cst.tile([128, 6], F32)
            nc.sync.dma_start(vfix[:], vfix_d[:])
            w1 = cst.tile([FV, H1], F32)
            nc.sync.dma_start(w1[:], w1_d[:])
            wl_sb = {}
            pv_sb = {}
            bl_sb = {}
            for nm, wd, pvd, bd in layers:
                wl_sb[nm] = cst.tile([FE, FE], F32, tag=f"w{nm}", name=f"w{nm}sb")
                nc.sync.dma_start(wl_sb[nm][:], wd[:])
                pv_sb[nm] = cst.tile([128, 2], F32, tag=f"pv{nm}", name=f"pv{nm}sb")
                nc.sync.dma_start(pv_sb[nm][:], pvd[:])
                bl_sb[nm] = cst.tile([FE, 1], F32, tag=f"b{nm}", name=f"b{nm}sb")
                nc.sync.dma_start(bl_sb[nm][:], bd[:])
            w4 = cst.tile([128, 2, H2], F32)
            nc.sync.dma_start(w4[:], w4_d[:])
            w5 = cst.tile([H2, NCLS], F32)
            nc.sync.dma_start(w5[:], w5_d[:])
            pv4 = cst.tile([FE, 1], F32)
            nc.sync.dma_start(pv4[:], pv4_d[:])
            b1 = cst.tile([128, 2], F32)
            nc.sync.dma_start(b1[:], b1_d[:])
            b4g = cst.tile([1, H2], F32)
            nc.sync.dma_start(b4g[:], b4g_d[:])
            b4 = cst.tile([H2, 1], F32)
            nc.sync.dma_start(b4[:], b4_d[:])
            b5 = cst.tile([NCLS, 1], F32)
            nc.sync.dma_start(b5[:], b5_d[:])

            # T slab (lhsT panel for m2 GEMM + U GEMM): [128, 12, 2, 752]
            tslab = cst.tile([128, NDR, 2, MRP], BF16)
            nc.vector.memset(tslab[:], 0.0)
            for dci in range(NDR):
                for j in range(2):
                    r0 = dci * 256 + j * 128
                    nc.sync.dma_start(tslab[:, dci, j, :MR], tslab_d[r0:r0 + 128, :])

            # Zh (transposed layout), updated per edge layer
            zht = cst.tile([FE, M], F32)
            nc.sync.dma_start(zht[:], zt_d[:])
            nc.scalar.activation(zht[:], zht[:], AF.Relu)

            # per-layer sv columns [128, 24] + local-Xh columns, se, etc.
            svcol = {nm: cst.tile([128, len(NCH)], F32, tag=f"sv{nm}", name=f"sv{nm}sb") for nm, *_ in layers}
            xh_loc = cst.tile([128, 2, NR], F32)     # core's own XhT columns
            xw4 = cst.tile([128, len(NLB), H2], F32)  # XW4 for core's node rows
            secol = cst.tile([128, len(IBL)], F32)
            cmax = cst.tile([128, len(KCH)], F32)
            rmax = cst.tile([128, 6016], F32)
            invc = cst.tile([128, len(KCH)], F32)
            enrelu = cst.tile([FE, MR], F32)          # core's relu'd EnT piece
            u_raw = cst.tile([128, len(IBL), H2], F32)  # T.T @ G (pre-se-scale)

            # DRAM scratch
            at_dram = [dram.tile([768, 6016], BF16, tag=f"at{i}", name=f"atdram{i}") for i in range(2)]
            sv_gin = dram.tile([3, NR], F32)
            sv_gout = dram.tile([CORES, 3, NR], F32)
            cm_in = dram.tile([128, len(KCH)], F32)
            cm_out = dram.tile([128, len(KCH)], F32)
            zg_in = dram.tile([FE, MR], F32)
            zg_out = dram.tile([CORES, FE, MR], F32)
            gd_in = dram.tile([N, H2], F32)
            gd_out = dram.tile([N, H2], F32)
            vt_in = dram.tile([H2, N], F32)
            vt_out = dram.tile([H2, N], F32)

            # ================= gc1 =================
            with (
                tc.tile_pool(name="g1", bufs=1) as g1,
                tc.tile_pool(name="g1s", bufs=3) as g1s,
                tc.tile_pool(name="g1p", bufs=2, space="PSUM") as g1p,
                tc.tile_pool(name="g1px", bufs=2, space="PSUM") as g1px,
            ):
                xt_sb = g1.tile([FV, N], F32)
                nc.sync.dma_start(xt_sb[:], xt_d[:])
                xw1 = g1.tile([128, len(NCH), H1], F32)
                for ci, (ns, nsz) in enumerate(NCH):
                    ps = g1p.tile([128, H1], F32)
                    nc.tensor.matmul(ps[:nsz, :], (xt_sb[:, ns:ns + nsz]), (w1[:]),
                                     start=True, stop=True)
                    nc.scalar.copy(xw1[:nsz, ci, :], ps[:nsz, :])
                psx = [g1px.tile([128, NR], F32, tag=f"psx{hb}", name=f"psx{hb}t") for hb in range(2)]
                for ci, (ns, nsz) in enumerate(NCH):
                    ptc = g1s.tile([128, NR], F32, tag="ptc")
                    nc.sync.dma_start(ptc[:nsz, :], ptcol_d[ns:ns + nsz, :])
                    for hb in range(2):
                        nc.tensor.matmul(
                            psx[hb][:, :],
                            (xw1[:nsz, ci, hb * 128:(hb + 1) * 128]),
                            (ptc[:nsz, :]),
                            start=(ci == 0), stop=(ci == len(NCH) - 1))
                for hb in range(2):
                    nc.scalar.activation(xh_loc[:, hb, :], psx[hb][:, :], AF.Relu,
                                         bias=b1[:, hb:hb + 1])

            # ============ prep: local sv pieces -> tiny AllGather; XW4 local ============
            with (
                tc.tile_pool(name="prep", bufs=1) as pr,
                tc.tile_pool(name="prp", bufs=2, space="PSUM") as prp,
            ):
                svp = pr.tile([128, 3, len(NLB)], F32)
                for li, (nm, _, _, _) in enumerate(layers):
                    for j, (nl, nlsz) in enumerate(NLB):
                        ps = prp.tile([128, 1], F32, tag="psv")
                        for hb in range(2):
                            nc.tensor.matmul(ps[:nlsz, :], xh_loc[:, hb, nl:nl + nlsz],
                                             pv_sb[nm][:, hb:hb + 1],
                                             start=(hb == 0), stop=(hb == 1))
                        nc.scalar.copy(svp[:nlsz, li, j:j + 1], ps[:nlsz, :])
                        nc.sync.dma_start(sv_gin[li, nl:nl + nlsz],
                                          svp[:nlsz, li, j:j + 1])
                nc.gpsimd.collective_compute(
                    "AllGather", ALU.bypass, replica_groups=RG,
                    ins=[sv_gin[:].opt()], outs=[sv_gout[:].opt()])
                # scatter gathered sv values into chunk-major [128, 24] columns
                for li, (nm, _, _, _) in enumerate(layers):
                    for ci, (ns, nsz) in enumerate(NCH):
                        lo = ns
                        while lo < ns + nsz:
                            r = lo // NR
                            take = min((r + 1) * NR, ns + nsz) - lo
                            nc.sync.dma_start(
                                svcol[nm][lo - ns:lo - ns + take, ci:ci + 1],
                                sv_gout[r, li, lo - r * NR:lo - r * NR + take]
                                .unsqueeze(-1))
                            lo += take
                for j, (nl, nlsz) in enumerate(NLB):
                    ps = prp.tile([128, H2], F32, tag="psw4")
                    for hb in range(2):
                        nc.tensor.matmul(ps[:nlsz, :], xh_loc[:, hb, nl:nl + nlsz],
                                         w4[:, hb, :], start=(hb == 0), stop=(hb == 1))
                    nc.scalar.copy(xw4[:nlsz, j, :], ps[:nlsz, :])

            # ===== gc4 early: G = P @ XW4 + b4g and U = T.T @ G (independent of Zh) =====
            with (
                tc.tile_pool(name="g4e", bufs=1) as g4e,
                tc.tile_pool(name="g4es", bufs=3) as g4es,
                tc.tile_pool(name="g4ep", bufs=2, space="PSUM") as g4ep,
                tc.tile_pool(name="g4ep1", bufs=1, space="PSUM") as g4ep1,
            ):
                for ci, (ns, nsz) in enumerate(NCH):
                    ps = g4ep.tile([128, H2], F32, tag="psg")
                    for j, (nl, nlsz) in enumerate(NLB):
                        ptr = g4es.tile([128, 128], F32, tag="ptr")
                        nc.sync.dma_start(ptr[:nlsz, :nsz], ptrow_d[nl:nl + nlsz, ns:ns + nsz])
                        nc.tensor.matmul(ps[:nsz, :], ptr[:nlsz, :nsz], xw4[:nlsz, j, :],
                                         start=(j == 0), stop=(j == len(NLB) - 1))
                    gst = g4es.tile([128, H2], F32, tag="gst")
                    nc.scalar.copy(gst[:nsz, :], ps[:nsz, :])
                    nc.sync.dma_start(gd_in[ns:ns + nsz, :], gst[:nsz, :])
                nc.gpsimd.collective_compute(
                    "AllReduce", ALU.add, replica_groups=RG,
                    ins=[gd_in[:].opt()], outs=[gd_out[:].opt()])

            # ================= edge layers (software-pipelined emission) =================
            with (
                tc.tile_pool(name="ts", bufs=20) as tsp,
                tc.tile_pool(name="aux", bufs=3) as auxp,
                tc.tile_pool(name="abuf", bufs=3) as abufp,
                tc.tile_pool(name="ats", bufs=3) as atsp,
                tc.tile_pool(name="fix", bufs=2) as fixp,
                tc.tile_pool(name="hew", bufs=1) as hewp,
                tc.tile_pool(name="tsvp", bufs=2) as tsvp,
                tc.tile_pool(name="m2p", bufs=3, space="PSUM") as m2p,
                tc.tile_pool(name="tpp", bufs=1, space="PSUM") as tpp,
                tc.tile_pool(name="hwp", bufs=1, space="PSUM") as hwp,
                tc.tile_pool(name="enp", bufs=1, space="PSUM") as enp,
                tc.tile_pool(name="gup", bufs=1) as gup,
            ):
                vfixb = hewp.tile([128, len(IBL), 512], F32, tag="vfixb")
                for ib in range(len(IBL)):
                    nc.vector.tensor_scalar(vfixb[:, ib, :], ones512[:],
                                            vfix[:, ib:ib + 1], None, op0=ALU.mult)
                tsv_t = {}
                hew_t = {}

                def emit_g_and_u():
                    ones1 = gup.tile([1, 128], F32)
                    nc.vector.memset(ones1[:], 1.0)
                    psb = hwp.tile([128, H2], F32, tag="pshew", name="psb4g")
                    nc.tensor.matmul(psb[:, :], ones1[:, :], b4g[:, :],
                                     start=True, stop=True)
                    b4gb = gup.tile([128, H2], F32)
                    nc.scalar.copy(b4gb[:], psb[:, :])
                    g_sb = gup.tile([128, len(NCH), H2], BF16)
                    for ci, (ns, nsz) in enumerate(NCH):
                        gch = fixp.tile([128, H2], F32, tag="gch", name=f"gch{ci}")
                        nc.sync.dma_start(gch[:nsz, :], gd_out[ns:ns + nsz, :])
                        nc.vector.tensor_add(g_sb[:nsz, ci, :], gch[:nsz, :],
                                             b4gb[:nsz, :])
                    for kb, (k0, ksz) in enumerate(IBL):
                        ps = hwp.tile([128, H2], F32, tag="pshew", name=f"psu{kb}")
                        for ci, (ns, nsz) in enumerate(NCH):
                            nc.tensor.matmul(ps[:ksz, :],
                                             tslab[:nsz, ci // 2, ci % 2, k0:k0 + ksz],
                                             g_sb[:nsz, ci, :],
                                             start=(ci == 0), stop=(ci == len(NCH) - 1))
                        nc.scalar.copy(u_raw[:ksz, kb, :], ps[:ksz, :])

                def emit_tsv(li):
                    nm = layers[li][0]
                    t = tsvp.tile([128, NDR, 2, MRP], F8, tag="tsv", name=f"tsv{li}")
                    tsv_t[li] = t
                    for ci in range(len(NCH)):
                        nc.scalar.activation(t[:, ci // 2, ci % 2, :],
                                             tslab[:, ci // 2, ci % 2, :], AF.Copy,
                                             scale=svcol[nm][:, ci:ci + 1])
                    nc.vector.memset(rmax[:], -3.0e38)

                def emit_pass1_kb(li, kb):
                    atd = at_dram[li % 2]
                    tsv = tsv_t[li]
                    k0, kbs = KBL[kb]
                    ts_tiles = []
                    for dci in range(NDR):
                        tst = tsp.tile([128, 2, 512], F8, tag="ts", name=f"ts{li}_{kb}_{dci}")
                        nc.sync.dma_start(
                            tst[:, :, :kbs],
                            tpad_d[dci * 256:(dci + 1) * 256, k0:k0 + kbs]
                            .rearrange("(two p) k -> p two k", p=128))
                        ts_tiles.append(tst)
                    for ib, (i0, ibs) in enumerate(IBL):
                        ibp = (ibs + 3) // 4 * 4
                        pm = m2p.tile([128, 512], F32, tag="pm", name=f"pm{li}_{kb}_{ib}")
                        for dci in range(NDR):
                            nc.tensor.matmul(
                                pm[:ibp, :kbs], tsv[:, dci, :, i0:i0 + ibp],
                                ts_tiles[dci][:, :, :kbs],
                                start=(dci == 0), stop=(dci == NDR - 1),
                                perf_mode=PM_DR)
                        adj = auxp.tile([128, 512], F32, tag="aux", name=f"adj{li}_{kb}_{ib}")
                        nc.sync.dma_start(adj[:ibs, :kbs],
                                          adjslab_d[i0:i0 + ibs, k0:k0 + kbs])
                        asb = abufp.tile([128, 512], F32, tag="a", name=f"a{li}_{kb}_{ib}")
                        nc.vector.tensor_mul(asb[:ibs, :kbs], pm[:ibs, :kbs],
                                             adj[:ibs, :kbs])
                        ck = fixp.tile([128, 1], F32, tag="ck", name=f"ck{li}_{kb}_{ib}")
                        nc.vector.tensor_scalar(ck[:ibs, :], cidx[:ibs, ib:ib + 1],
                                                float(-k0), None, op0=ALU.add)
                        mk = fixp.tile([128, 512], mybir.dt.uint8, tag="mk",
                                       name=f"mk{li}_{kb}_{ib}")
                        nc.vector.tensor_scalar(mk[:ibs, :kbs], kiota[:ibs, :kbs],
                                                ck[:ibs, :], None, op0=ALU.is_equal)
                        nc.vector.copy_predicated(asb[:ibs, :kbs], mk[:ibs, :kbs],
                                                  vfixb[:ibs, ib, :kbs])
                        nc.vector.tensor_max(rmax[:ibs, k0:k0 + kbs],
                                             rmax[:ibs, k0:k0 + kbs], asb[:ibs, :kbs])
                        ats = atsp.tile([128, 512], BF16, tag="ats", name=f"ats{li}_{kb}_{ib}")
                        nc.scalar.copy(ats[:ibs, :kbs], asb[:ibs, :kbs])
                        nc.sync.dma_start(atd[i0:i0 + ibs, k0:k0 + kbs], ats[:ibs, :kbs])

                def emit_cmax_ar(li):
                    for kc, (k0, kcs) in enumerate(KCH):
                        tp = tpp.tile([128, 128], F32, tag="tp", name=f"tpr{li}_{kc}")
                        nc.tensor.transpose(tp[:kcs, :], rmax[:, k0:k0 + kcs], ident[:, :])
                        nc.vector.reduce_max(cmax[:kcs, kc:kc + 1], tp[:kcs, :],
                                             axis=mybir.AxisListType.X)
                    nc.sync.dma_start(cm_in[:], cmax[:])
                    nc.gpsimd.collective_compute(
                        "AllReduce", ALU.max, replica_groups=RG,
                        ins=[cm_in[:].opt()], outs=[cm_out[:].opt()])

                def emit_hew(li):
                    nm = layers[li][0]
                    hw = hewp.tile([128, len(KCH), FE], BF16, tag="hew", name=f"hew{li}")
                    hew_t[li] = hw
                    for kc, (k0, kcs) in enumerate(KCH):
                        ps = hwp.tile([128, FE], F32, tag="pshew", name=f"ph{li}_{kc}")
                        nc.tensor.matmul(ps[:kcs, :], zht[:, k0:k0 + kcs], wl_sb[nm][:],
                                         start=True, stop=True)
                        nc.scalar.copy(hw[:kcs, kc, :], ps[:kcs, :])

                def emit_scale_hew(li):
                    hw = hew_t[li]
                    nc.sync.dma_start(cmax[:], cm_out[:])
                    nc.vector.reciprocal(invc[:], cmax[:])
                    for kc, (k0, kcs) in enumerate(KCH):
                        nc.vector.tensor_scalar_mul(hw[:kcs, kc, :], hw[:kcs, kc, :],
                                                    invc[:kcs, kc:kc + 1])

                def emit_pass2(li, last):
                    nm, _, _, bd = layers[li]
                    atd = at_dram[li % 2]
                    hw = hew_t[li]
                    pes = [enp.tile([FE, 384], F32, tag=f"pe{fb}", name=f"pe{li}_{fb}")
                           for fb in range(2)]
                    for kc, (k0, kcs) in enumerate(KCH):
                        att = auxp.tile([128, 768], BF16, tag="attb",
                                        name=f"att{li}_{kc}")
                        eng = nc.sync if kc % 2 == 0 else nc.scalar
                        eng.dma_start_transpose(
                            att[:, :], atd[0:768, kc * 128:(kc + 1) * 128])
                        for fb, f0 in enumerate((0, 384)):
                            nc.tensor.matmul(pes[fb][:, :], hw[:kcs, kc, :],
                                             att[:kcs, f0:f0 + 384],
                                             start=(kc == 0), stop=(kc == len(KCH) - 1))
                    for fb, f0 in enumerate((0, 384)):
                        fvalid = 384 if fb == 0 else MR - 384
                        nc.scalar.activation(enrelu[:, f0:f0 + fvalid],
                                             pes[fb][:, :fvalid],
                                             AF.Relu, bias=bl_sb[nm][:])
                    if not last:
                        nc.sync.dma_start(zg_in[:], enrelu[:])
                        nc.gpsimd.collective_compute(
                            "AllGather", ALU.bypass, replica_groups=RG,
                            ins=[zg_in[:].opt()], outs=[zg_out[:].opt()])
                        for r in range(CORES):
                            nc.sync.dma_start(zht[:, r * MR:(r + 1) * MR], zg_out[r, :, :])

                # pipelined emission: next layer's pass1 covers this layer's colmax AR
                emit_tsv(0)
                for kb in range(6):
                    emit_pass1_kb(0, kb)
                emit_g_and_u()
                for kb in range(6, len(KBL)):
                    emit_pass1_kb(0, kb)
                emit_cmax_ar(0)
                emit_hew(0)
                emit_tsv(1)
                for kb in range(3):
                    emit_pass1_kb(1, kb)
                emit_scale_hew(0)
                emit_pass2(0, last=False)
                for kb in range(3, len(KBL)):
                    emit_pass1_kb(1, kb)
                emit_cmax_ar(1)
                emit_hew(1)
                emit_tsv(2)
                for kb in range(3):
                    emit_pass1_kb(2, kb)
                emit_scale_hew(1)
                emit_pass2(1, last=False)
                for kb in range(3, len(KBL)):
                    emit_pass1_kb(2, kb)
                emit_cmax_ar(2)
                emit_hew(2)
                emit_scale_hew(2)
                emit_pass2(2, last=True)

            # ================= gc4 =================
            with (
                tc.tile_pool(name="g4", bufs=1) as g4,
                tc.tile_pool(name="g4s", bufs=3) as g4s,
                tc.tile_pool(name="g4p", bufs=2, space="PSUM") as g4p,
                tc.tile_pool(name="g4p1", bufs=1, space="PSUM") as g4p1,
            ):
                # se from the core's own relu'd Zh piece
                for kb, (k0, ksz) in enumerate(IBL):
                    ps = g4p1.tile([128, 1], F32, tag="small")
                    nc.tensor.matmul(ps[:ksz, :], enrelu[:, k0:k0 + ksz], pv4[:],
                                     start=True, stop=True)
                    nc.scalar.copy(secol[:ksz, kb:kb + 1], ps[:ksz, :])
                # scale precomputed U by se
                u_sb = g4.tile([128, len(IBL), H2], BF16)
                for kb, (k0, ksz) in enumerate(IBL):
                    nc.vector.tensor_scalar_mul(u_sb[:ksz, kb, :], u_raw[:ksz, kb, :],
                                                secol[:ksz, kb:kb + 1])
                # VT partial = (U*se).T-accum over local edge rows x TT slab
                for vb, (v0, vsz) in enumerate(VFB):
                    ps = g4p.tile([H2, 512], F32, tag="psvt")
                    for kb, (k0, ksz) in enumerate(IBL):
                        ttt = g4s.tile([128, 512], BF16, tag="ttt")
                        nc.sync.dma_start(ttt[:ksz, :vsz], ttslab_d[k0:k0 + ksz, v0:v0 + vsz])
                        nc.tensor.matmul(ps[:, :vsz], (u_sb[:ksz, kb, :]),
                                         (ttt[:ksz, :vsz]),
                                         start=(kb == 0), stop=(kb == len(IBL) - 1))
                    vst = g4s.tile([H2, 512], F32, tag="vst")
                    nc.scalar.copy(vst[:, :vsz], ps[:, :vsz])
                    nc.sync.dma_start(vt_in[:, v0:v0 + vsz], vst[:, :vsz])
                nc.gpsimd.collective_compute(
                    "AllReduce", ALU.add, replica_groups=RG,
                    ins=[vt_in[:].opt()], outs=[vt_out[:].opt()])

            # ================= gc5 + softmax =================
            with (
                tc.tile_pool(name="g5", bufs=1) as g5,
                tc.tile_pool(name="g5s", bufs=3) as g5s,
                tc.tile_pool(name="g5p", bufs=2, space="PSUM") as g5p,
                tc.tile_pool(name="g5pt", bufs=1, space="PSUM") as g5pt,
            ):
                xh5t = g5.tile([H2, N], F32)
                nc.sync.dma_start(xh5t[:], vt_out[:])
                nc.scalar.activation(xh5t[:], xh5t[:], AF.Relu, bias=b4[:])
                xw5 = g5.tile([128, len(NCH), NCLS], F32)
                for ci, (ns, nsz) in enumerate(NCH):
                    ps = g5p.tile([128, NCLS], F32, tag="psw5")
                    nc.tensor.matmul(ps[:nsz, :], xh5t[:, ns:ns + nsz], w5[:],
                                     start=True, stop=True)
                    nc.scalar.copy(xw5[:nsz, ci, :], ps[:nsz, :])
                pst = g5pt.tile([NCLS, 375], F32)
                for ci, (ns, nsz) in enumerate(NCH):
                    ptc = g5s.tile([128, NR], F32, tag="ptc5")
                    nc.sync.dma_start(ptc[:nsz, :], ptcol_d[ns:ns + nsz, :])
                    nc.tensor.matmul(pst[:, :], (xw5[:nsz, ci, :]), (ptc[:nsz, :]),
                                     start=(ci == 0), stop=(ci == len(NCH) - 1))
                st_sb = g5.tile([NCLS, NR], F32)
                nc.vector.tensor_scalar_add(st_sb[:], pst[:, :], b5[:])
                outt = g5.tile([128, len(NLB), NCLS], F32)
                ptp = g5pt.tile([128, len(NLB), NCLS], F32)
                for j, (t0, tsz) in enumerate(NLB):
                    nc.tensor.transpose(ptp[:tsz, j, :], st_sb[:, t0:t0 + tsz],
                                        ident[:NCLS, :NCLS])
                    red = g5s.tile([128, 1], F32, tag="red5")
                    nc.vector.reduce_max(red[:tsz, :], ptp[:tsz, j, :],
                                         axis=mybir.AxisListType.X)
                    nc.vector.tensor_scalar_mul(red[:tsz, :], red[:tsz, :], -1.0)
                    nc.scalar.activation(outt[:tsz, j, :], ptp[:tsz, j, :], AF.Exp,
                                         bias=red[:tsz, :])
                    ssum = g5s.tile([128, 1], F32, tag="ssum5")
                    nc.vector.reduce_sum(ssum[:tsz, :], outt[:tsz, j, :],
                                         axis=mybir.AxisListType.X)
                    nc.vector.reciprocal(ssum[:tsz, :], ssum[:tsz, :])
                    nc.vector.tensor_scalar_mul(outt[:tsz, j, :], outt[:tsz, j, :],
                                                ssum[:tsz, :])
                    nc.sync.dma_start(out_d[t0:t0 + tsz, :], outt[:tsz, j, :])

    nc.finalize()
    return nc


def prepare_inputs(inputs):
    f = lambda x: np.ascontiguousarray(np.asarray(x), dtype=np.float32)
    X, Z, adj_e, T = f(inputs["X"]), f(inputs["Z"]), f(inputs["adj_e"]), f(inputs["T"])
    ei = np.asarray(inputs["edge_index"])
    W1, b1 = f(inputs["W1"]), f(inputs["b1"])
    p2, W2, b2 = f(inputs["p2"]), f(inputs["W2"]), f(inputs["b2"])
    p3, W3, b3 = f(inputs["p3"]), f(inputs["W3"]), f(inputs["b3"])
    p32, W32, b32 = f(inputs["p32"]), f(inputs["W32"]), f(inputs["b32"])
    p4, W4 = f(inputs["p4"]), f(inputs["W4"])
    b4g, b4, W5, b5 = f(inputs["b4g"]), f(inputs["b4"]), f(inputs["W5"]), f(inputs["b5"])

    # dense PT = P.T where P is the symmetric-normalized (A+I) propagation matrix
    src = ei[0].astype(np.int64)
    dst = ei[1].astype(np.int64)
    loop = np.arange(N, dtype=np.int64)
    s = np.concatenate([src, loop])
    d = np.concatenate([dst, loop])
    deg = np.zeros(N, np.float32)
    np.add.at(deg, d, np.float32(1.0))
    dinv = np.where(deg > 0, 1.0 / np.sqrt(deg), 0.0).astype(np.float32)
    norm = dinv[s] * dinv[d]
    PT = np.zeros((N, N), np.float32)
    np.add.at(PT, (s, d), norm)

    import ml_dtypes
    bf16 = ml_dtypes.bfloat16
    fp8 = ml_dtypes.float8_e4m3
    Tpad = np.zeros((3072, M), np.float32)
    Tpad[:N] = T
    kiota = np.tile(np.arange(512, dtype=np.float32), (128, 1))
    base = dict(
        tpad=np.ascontiguousarray(Tpad.astype(fp8)),
        xt=np.ascontiguousarray(X.T), zt=np.ascontiguousarray(Z.T),
        kiota=kiota, w1=W1, w2=W2, w3=W3, w32=W32,
        w4=np.ascontiguousarray(np.transpose(W4.reshape(2, 128, H2), (1, 0, 2))),
        w5=W5,
        pv2=np.ascontiguousarray(p2[0].reshape(2, 128).T),
        pv3=np.ascontiguousarray(p3[0].reshape(2, 128).T),
        pv32=np.ascontiguousarray(p32[0].reshape(2, 128).T),
        pv4=np.ascontiguousarray(p4[0][:, None]),
        b1=np.ascontiguousarray(b1.reshape(2, 128).T),
        b2=b2[:, None], b3=b3[:, None], b32=b32[:, None],
        b4g=b4g[None, :], b4=b4[:, None], b5=b5[:, None],
    )
    in_maps = []
    for c in range(CORES):
        e0, n0 = c * MR, c * NR
        m = dict(base)
        m["tslab"] = np.ascontiguousarray(Tpad[:, e0:e0 + MR].astype(bf16))
        m["ttslab"] = np.ascontiguousarray(T[:, e0:e0 + MR].T.astype(bf16))
        m["adjslab"] = np.ascontiguousarray(adj_e[e0:e0 + MR, :])
        m["ptcol"] = np.ascontiguousarray(PT[:, n0:n0 + NR])
        m["ptrow"] = np.ascontiguousarray(PT[n0:n0 + NR, :])
        m["cidx"] = _col_layout(np.arange(e0, e0 + MR, dtype=np.float32), 128)
        m["cidx"][110:, 5] = -1.0  # pad slots beyond row 750 must never match
        m["vfix"] = _col_layout(np.diagonal(adj_e[e0:e0 + MR, e0:e0 + MR]).astype(np.float32), 128)
        in_maps.append({k: (np.ascontiguousarray(v) if v.dtype in (bf16, fp8)
                            else np.ascontiguousarray(v, dtype=np.float32))
                        for k, v in m.items()})
    return in_maps


_CACHE = {}


def kernel(**inputs):
    in_maps = prepare_inputs(inputs)
    if "nc" not in _CACHE:
        _CACHE["nc"] = build_program()
    res = bass_utils.run_bass_kernel_spmd(_CACHE["nc"], in_maps, list(range(CORES)))
    out = np.concatenate([res.results[c]["out"] for c in range(CORES)], axis=0)
    return out.astype(np.float32)


if __name__ == "__main__":
    import reference
    ins = reference.setup_inputs()
    ins = {k: np.asarray(v) for k, v in ins.items()}
    got = kernel(**ins)
    print("kernel output", got.shape, got.dtype)


# revision 26
# speedup vs baseline: 1.3122x; 1.0203x over previous
"""CensNet Trainium2 kernel — 8-core SPMD Bass/Tile implementation.

Computation (reference semantics):
  gc1: Xh = relu(P @ (X @ W1) + b1)   with P = sym-normalized (A+I) from edge_index
  Zh = relu(Z)
  3x edge layers (p2/W2, p3/W3, p32/W32):
      sv = Xh @ p.T                      [n]
      m2 = T.T @ (T * sv[:,None])        [m,m]  (symmetric)
      A  = (m2 with diag<-1) * adj_e
      Zh = relu(A @ (Zh@W / colmax(A)) + b)     (colmax folded into right operand)
  gc4: se = Zh @ p4.T; X4 = (T*se) @ (T.T @ (P @ (Xh@W4) + b4g)) + b4   (associativity;
       never materializes the [n,n] multiplier)
  gc5: softmax(P @ (relu(X4) @ W5) + b5, axis=1)

Sharding: edge dim m row-sharded 8 ways (750 rows/core) for all m x m work;
node dim n sharded 375 rows/core for P-products. Weights replicated. Collectives:
AllGather (XhT, ZhT pieces), AllReduce (colmax max, G, VT sums).
"""

import sys

for _p in ("/opt/trn_rl_repo", "/root/.axon_site/_ro/trn_rl_repo"):
    if _p not in sys.path:
        sys.path.insert(0, _p)

import numpy as np

import concourse.bass as bass
import concourse.mybir as mybir
import concourse.tile as tile
from concourse import bacc, bass_utils
from concourse.masks import make_identity

F32 = mybir.dt.float32
BF16 = mybir.dt.bfloat16
F8 = mybir.dt.float8e4
PM_DR = mybir.MatmulPerfMode.DoubleRow
AF = mybir.ActivationFunctionType
ALU = mybir.AluOpType

CORES = 8
N, M = 3000, 6000
FV, FE, H1, H2, NCLS = 128, 64, 256, 128, 16
MR = M // CORES   # 750 edge rows per core
MRP = 752         # fp8 LDW needs 4-byte-aligned plane stride
NR = N // CORES   # 375 node rows per core


def _chunks(total, step):
    return [(s, min(step, total - s)) for s in range(0, total, step)]


NCH = _chunks(N, 128)      # 24 contraction chunks over n
NPAD = 3072                # n padded to 12 DoubleRow chunks of 256
NDR = 12
KCH = _chunks(M, 128)      # 47 chunks over m (pass-2 contraction)
KBL = _chunks(M, 512)      # 12 streaming blocks over m
IBL = _chunks(MR, 128)     # 6 row blocks within the core's 750 rows
IFB = _chunks(MR, 375)     # 2 free blocks for pass-2 output
NLB = _chunks(NR, 128)     # 3 local node blocks
VFB = _chunks(N, 512)      # 6 free blocks over n for the VT GEMM
RG = [list(range(CORES))]


def _col_layout(vec, p=128):
    """[L] -> [p, ceil(L/p)] chunk-major (column j holds vec[j*p:(j+1)*p])."""
    L = len(vec)
    ncol = (L + p - 1) // p
    out = np.zeros((p, ncol), np.float32)
    for j in range(ncol):
        seg = vec[j * p:(j + 1) * p]
        out[: len(seg), j] = seg
    return out


def build_program():
    nc = bacc.Bacc("TRN2", target_bir_lowering=False, debug=False,
                   num_devices=CORES)

    dp = lambda name, shape, dt=F32: nc.declare_dram_parameter(name, list(shape), dt, isOutput=False)
    tpad_d = dp("tpad", (NPAD, M), F8)
    tslab_d = dp("tslab", (NPAD, MR), BF16)
    ttslab_d = dp("ttslab", (MR, N), BF16)
    adjslab_d = dp("adjslab", (MR, M))
    ptcol_d = dp("ptcol", (N, NR), BF16)
    ptrow_d = dp("ptrow", (NR, N))
    xt_d = dp("xt", (FV, N), BF16)
    zt_d = dp("zt", (FE, M))
    kiota_d = dp("kiota", (128, 512))
    cidx_d = dp("cidx", (128, 6))
    vfix_d = dp("vfix", (128, 6))
    w1_d = dp("w1", (FV, H1), BF16)
    w2_d = dp("w2", (FE, FE))
    w3_d = dp("w3", (FE, FE))
    w32_d = dp("w32", (FE, FE))
    w4_d = dp("w4", (128, 2, H2))
    w5_d = dp("w5", (H2, NCLS), BF16)
    pv2_d = dp("pv2", (128, 2))
    pv3_d = dp("pv3", (128, 2))
    pv32_d = dp("pv32", (128, 2))
    pv4_d = dp("pv4", (FE, 1))
    b1_d = dp("b1", (128, 2))
    b2_d = dp("b2", (FE, 1))
    b3_d = dp("b3", (FE, 1))
    b32_d = dp("b32", (FE, 1))
    b4g_d = dp("b4g", (1, H2))
    b4_d = dp("b4", (H2, 1))
    b5_d = dp("b5", (NCLS, 1))
    out_d = nc.declare_dram_parameter("out", [NR, NCLS], F32, isOutput=True)

    layers = [("2", w2_d, pv2_d, b2_d), ("3", w3_d, pv3_d, b3_d), ("32", w32_d, pv32_d, b32_d)]

    with tile.TileContext(nc) as tc:
        with (
            tc.tile_pool(name="const", bufs=1) as cst,
            tc.tile_pool(name="dram", bufs=1, space="DRAM") as dram,
        ):
            # ------- persistent constants / state -------
            ident = cst.tile([128, 128], F32)
            make_identity(nc, ident[:])
            ones512 = cst.tile([128, 512], F32)
            nc.vector.memset(ones512[:], 1.0)
            kiota = cst.tile([128, 512], F32)
            nc.sync.dma_start(kiota[:], kiota_d[:])
            cidx = cst.tile([128, 6], F32)
            nc.sync.dma_start(cidx[:], cidx_d[:])
            vfix = cst.tile([128, 6], F32)
            nc.sync.dma_start(vfix[:], vfix_d[:])
            w1 = cst.tile([FV, H1], BF16)
            nc.sync.dma_start(w1[:], w1_d[:])
            wl_sb = {}
            pv_sb = {}
            bl_sb = {}
            for nm, wd, pvd, bd in layers:
                wl_sb[nm] = cst.tile([FE, FE], F32, tag=f"w{nm}", name=f"w{nm}sb")
                nc.sync.dma_start(wl_sb[nm][:], wd[:])
                pv_sb[nm] = cst.tile([128, 2], F32, tag=f"pv{nm}", name=f"pv{nm}sb")
                nc.sync.dma_start(pv_sb[nm][:], pvd[:])
                bl_sb[nm] = cst.tile([FE, 1], F32, tag=f"b{nm}", name=f"b{nm}sb")
                nc.sync.dma_start(bl_sb[nm][:], bd[:])
            w4 = cst.tile([128, 2, H2], F32)
            nc.sync.dma_start(w4[:], w4_d[:])
            w5 = cst.tile([H2, NCLS], BF16)
            nc.sync.dma_start(w5[:], w5_d[:])
            pv4 = cst.tile([FE, 1], F32)
            nc.sync.dma_start(pv4[:], pv4_d[:])
            b1 = cst.tile([128, 2], F32)
            nc.sync.dma_start(b1[:], b1_d[:])
            b4g = cst.tile([1, H2], F32)
            nc.sync.dma_start(b4g[:], b4g_d[:])
            b4 = cst.tile([H2, 1], F32)
            nc.sync.dma_start(b4[:], b4_d[:])
            b5 = cst.tile([NCLS, 1], F32)
            nc.sync.dma_start(b5[:], b5_d[:])

            # T slab (lhsT panel for m2 GEMM + U GEMM): [128, 12, 2, 752]
            tslab = cst.tile([128, NDR, 2, MRP], BF16)
            nc.vector.memset(tslab[:], 0.0)
            for dci in range(NDR):
                for j in range(2):
                    r0 = dci * 256 + j * 128
                    nc.sync.dma_start(tslab[:, dci, j, :MR], tslab_d[r0:r0 + 128, :])

            # Zh (transposed layout), updated per edge layer
            zht = cst.tile([FE, M], F32)
            nc.sync.dma_start(zht[:], zt_d[:])
            nc.scalar.activation(zht[:], zht[:], AF.Relu)

            # per-layer sv columns [128, 24] + local-Xh columns, se, etc.
            svcol = {nm: cst.tile([128, len(NCH)], F32, tag=f"sv{nm}", name=f"sv{nm}sb") for nm, *_ in layers}
            xh_loc = cst.tile([128, 2, NR], F32)     # core's own XhT columns
            xw4 = cst.tile([128, len(NLB), H2], F32)  # XW4 for core's node rows
            secol = cst.tile([128, len(IBL)], F32)
            cmax = cst.tile([128, len(KCH)], F32)
            rmax = cst.tile([128, 6016], F32)
            invc = cst.tile([128, len(KCH)], F32)
            enrelu = cst.tile([FE, MR], F32)          # core's relu'd EnT piece
            u_raw = cst.tile([128, len(IBL), H2], F32)  # T.T @ G (pre-se-scale)

            # DRAM scratch
            at_dram = [dram.tile([768, 6016], BF16, tag=f"at{i}", name=f"atdram{i}") for i in range(2)]
            sv_gin = dram.tile([3, NR], F32)
            sv_gout = dram.tile([CORES, 3, NR], F32)
            cm_in = dram.tile([128, len(KCH)], F32)
            cm_out = dram.tile([128, len(KCH)], F32)
            zg_in = dram.tile([FE, MR], F32)
            zg_out = dram.tile([CORES, FE, MR], F32)
            gd_in = dram.tile([N, H2], BF16)
            gd_out = dram.tile([N, H2], BF16)
            vt_in = dram.tile([H2, N], BF16)
            vt_out = dram.tile([H2, N], BF16)

            # ================= gc1 =================
            with (
                tc.tile_pool(name="g1", bufs=1) as g1,
                tc.tile_pool(name="g1s", bufs=3) as g1s,
                tc.tile_pool(name="g1p", bufs=2, space="PSUM") as g1p,
                tc.tile_pool(name="g1px", bufs=2, space="PSUM") as g1px,
            ):
                xt_sb = g1.tile([FV, N], BF16)
                nc.sync.dma_start(xt_sb[:], xt_d[:])
                xw1 = g1.tile([128, len(NCH), H1], BF16)
                for ci, (ns, nsz) in enumerate(NCH):
                    ps = g1p.tile([128, H1], F32)
                    nc.tensor.matmul(ps[:nsz, :], (xt_sb[:, ns:ns + nsz]), (w1[:]),
                                     start=True, stop=True)
                    nc.scalar.copy(xw1[:nsz, ci, :], ps[:nsz, :])
                psx = [g1px.tile([128, NR], F32, tag=f"psx{hb}", name=f"psx{hb}t") for hb in range(2)]
                for ci, (ns, nsz) in enumerate(NCH):
                    ptc = g1s.tile([128, NR], BF16, tag="ptc")
                    nc.sync.dma_start(ptc[:nsz, :], ptcol_d[ns:ns + nsz, :])
                    for hb in range(2):
                        nc.tensor.matmul(
                            psx[hb][:, :],
                            (xw1[:nsz, ci, hb * 128:(hb + 1) * 128]),
                            (ptc[:nsz, :]),
                            start=(ci == 0), stop=(ci == len(NCH) - 1))
                for hb in range(2):
                    nc.scalar.activation(xh_loc[:, hb, :], psx[hb][:, :], AF.Relu,
                                         bias=b1[:, hb:hb + 1])

            # ============ prep: local sv pieces -> tiny AllGather; XW4 local ============
            with (
                tc.tile_pool(name="prep", bufs=1) as pr,
                tc.tile_pool(name="prp", bufs=2, space="PSUM") as prp,
            ):
                svp = pr.tile([128, 3, len(NLB)], F32)
                for li, (nm, _, _, _) in enumerate(layers):
                    for j, (nl, nlsz) in enumerate(NLB):
                        ps = prp.tile([128, 1], F32, tag="psv")
                        for hb in range(2):
                            nc.tensor.matmul(ps[:nlsz, :], xh_loc[:, hb, nl:nl + nlsz],
                                             pv_sb[nm][:, hb:hb + 1],
                                             start=(hb == 0), stop=(hb == 1))
                        nc.scalar.copy(svp[:nlsz, li, j:j + 1], ps[:nlsz, :])
                        nc.sync.dma_start(sv_gin[li, nl:nl + nlsz],
                                          svp[:nlsz, li, j:j + 1])
                nc.gpsimd.collective_compute(
                    "AllGather", ALU.bypass, replica_groups=RG,
                    ins=[sv_gin[:].opt()], outs=[sv_gout[:].opt()])
                # scatter gathered sv values into chunk-major [128, 24] columns
                for li, (nm, _, _, _) in enumerate(layers):
                    for ci, (ns, nsz) in enumerate(NCH):
                        lo = ns
                        while lo < ns + nsz:
                            r = lo // NR
                            take = min((r + 1) * NR, ns + nsz) - lo
                            nc.sync.dma_start(
                                svcol[nm][lo - ns:lo - ns + take, ci:ci + 1],
                                sv_gout[r, li, lo - r * NR:lo - r * NR + take]
                                .unsqueeze(-1))
                            lo += take
                for j, (nl, nlsz) in enumerate(NLB):
                    ps = prp.tile([128, H2], F32, tag="psw4")
                    for hb in range(2):
                        nc.tensor.matmul(ps[:nlsz, :], xh_loc[:, hb, nl:nl + nlsz],
                                         w4[:, hb, :], start=(hb == 0), stop=(hb == 1))
                    nc.scalar.copy(xw4[:nlsz, j, :], ps[:nlsz, :])

            # ===== gc4 early: G = P @ XW4 + b4g and U = T.T @ G (independent of Zh) =====
            with (
                tc.tile_pool(name="g4e", bufs=1) as g4e,
                tc.tile_pool(name="g4es", bufs=3) as g4es,
                tc.tile_pool(name="g4ep", bufs=2, space="PSUM") as g4ep,
                tc.tile_pool(name="g4ep1", bufs=1, space="PSUM") as g4ep1,
            ):
                for ci, (ns, nsz) in enumerate(NCH):
                    ps = g4ep.tile([128, H2], F32, tag="psg")
                    for j, (nl, nlsz) in enumerate(NLB):
                        ptr = g4es.tile([128, 128], F32, tag="ptr")
                        nc.sync.dma_start(ptr[:nlsz, :nsz], ptrow_d[nl:nl + nlsz, ns:ns + nsz])
                        nc.tensor.matmul(ps[:nsz, :], ptr[:nlsz, :nsz], xw4[:nlsz, j, :],
                                         start=(j == 0), stop=(j == len(NLB) - 1))
                    gst = g4es.tile([128, H2], BF16, tag="gst")
                    nc.scalar.copy(gst[:nsz, :], ps[:nsz, :])
                    nc.sync.dma_start(gd_in[ns:ns + nsz, :], gst[:nsz, :])
                nc.gpsimd.collective_compute(
                    "AllReduce", ALU.add, replica_groups=RG,
                    ins=[gd_in[:].opt()], outs=[gd_out[:].opt()])

            # ================= edge layers (software-pipelined emission) =================
            with (
                tc.tile_pool(name="ts", bufs=20) as tsp,
                tc.tile_pool(name="aux", bufs=3) as auxp,
                tc.tile_pool(name="abuf", bufs=3) as abufp,
                tc.tile_pool(name="ats", bufs=3) as atsp,
                tc.tile_pool(name="fix", bufs=2) as fixp,
                tc.tile_pool(name="hew", bufs=1) as hewp,
                tc.tile_pool(name="tsvp", bufs=2) as tsvp,
                tc.tile_pool(name="m2p", bufs=3, space="PSUM") as m2p,
                tc.tile_pool(name="tpp", bufs=1, space="PSUM") as tpp,
                tc.tile_pool(name="hwp", bufs=1, space="PSUM") as hwp,
                tc.tile_pool(name="enp", bufs=1, space="PSUM") as enp,
                tc.tile_pool(name="gup", bufs=1) as gup,
            ):
                vfixb = hewp.tile([128, len(IBL), 512], F32, tag="vfixb")
                for ib in range(len(IBL)):
                    nc.vector.tensor_scalar(vfixb[:, ib, :], ones512[:],
                                            vfix[:, ib:ib + 1], None, op0=ALU.mult)
                tsv_t = {}
                hew_t = {}

                def emit_g_and_u():
                    ones1 = gup.tile([1, 128], F32)
                    nc.vector.memset(ones1[:], 1.0)
                    psb = hwp.tile([128, H2], F32, tag="pshew", name="psb4g")
                    nc.tensor.matmul(psb[:, :], ones1[:, :], b4g[:, :],
                                     start=True, stop=True)
                    b4gb = gup.tile([128, H2], F32)
                    nc.scalar.copy(b4gb[:], psb[:, :])
                    g_sb = gup.tile([128, len(NCH), H2], BF16)
                    for ci, (ns, nsz) in enumerate(NCH):
                        gch = fixp.tile([128, H2], BF16, tag="gch", name=f"gch{ci}")
                        nc.sync.dma_start(gch[:nsz, :], gd_out[ns:ns + nsz, :])
                        nc.vector.tensor_add(g_sb[:nsz, ci, :], gch[:nsz, :],
                                             b4gb[:nsz, :])
                    for kb, (k0, ksz) in enumerate(IBL):
                        ps = hwp.tile([128, H2], F32, tag="pshew", name=f"psu{kb}")
                        for ci, (ns, nsz) in enumerate(NCH):
                            nc.tensor.matmul(ps[:ksz, :],
                                             tslab[:nsz, ci // 2, ci % 2, k0:k0 + ksz],
                                             g_sb[:nsz, ci, :],
                                             start=(ci == 0), stop=(ci == len(NCH) - 1))
                        nc.scalar.copy(u_raw[:ksz, kb, :], ps[:ksz, :])

                def emit_tsv(li):
                    nm = layers[li][0]
                    t = tsvp.tile([128, NDR, 2, MRP], F8, tag="tsv", name=f"tsv{li}")
                    tsv_t[li] = t
                    for ci in range(len(NCH)):
                        nc.scalar.activation(t[:, ci // 2, ci % 2, :],
                                             tslab[:, ci // 2, ci % 2, :], AF.Copy,
                                             scale=svcol[nm][:, ci:ci + 1])
                    nc.vector.memset(rmax[:], -3.0e38)

                def emit_pass1_kb(li, kb):
                    atd = at_dram[li % 2]
                    tsv = tsv_t[li]
                    k0, kbs = KBL[kb]
                    ts_tiles = []
                    for dci in range(NDR):
                        tst = tsp.tile([128, 2, 512], F8, tag="ts", name=f"ts{li}_{kb}_{dci}")
                        nc.sync.dma_start(
                            tst[:, :, :kbs],
                            tpad_d[dci * 256:(dci + 1) * 256, k0:k0 + kbs]
                            .rearrange("(two p) k -> p two k", p=128))
                        ts_tiles.append(tst)
                    for ib, (i0, ibs) in enumerate(IBL):
                        ibp = (ibs + 3) // 4 * 4
                        pm = m2p.tile([128, 512], F32, tag="pm", name=f"pm{li}_{kb}_{ib}")
                        for dci in range(NDR):
                            nc.tensor.matmul(
                                pm[:ibp, :kbs], tsv[:, dci, :, i0:i0 + ibp],
                                ts_tiles[dci][:, :, :kbs],
                                start=(dci == 0), stop=(dci == NDR - 1),
                                perf_mode=PM_DR)
                        adj = auxp.tile([128, 512], F32, tag="aux", name=f"adj{li}_{kb}_{ib}")
                        nc.sync.dma_start(adj[:ibs, :kbs],
                                          adjslab_d[i0:i0 + ibs, k0:k0 + kbs])
                        asb = abufp.tile([128, 512], F32, tag="a", name=f"a{li}_{kb}_{ib}")
                        nc.vector.tensor_mul(asb[:ibs, :kbs], pm[:ibs, :kbs],
                                             adj[:ibs, :kbs])
                        ck = fixp.tile([128, 1], F32, tag="ck", name=f"ck{li}_{kb}_{ib}")
                        nc.vector.tensor_scalar(ck[:ibs, :], cidx[:ibs, ib:ib + 1],
                                                float(-k0), None, op0=ALU.add)
                        mk = fixp.tile([128, 512], mybir.dt.uint8, tag="mk",
                                       name=f"mk{li}_{kb}_{ib}")
                        nc.vector.tensor_scalar(mk[:ibs, :kbs], kiota[:ibs, :kbs],
                                                ck[:ibs, :], None, op0=ALU.is_equal)
                        nc.vector.copy_predicated(asb[:ibs, :kbs], mk[:ibs, :kbs],
                                                  vfixb[:ibs, ib, :kbs])
                        nc.vector.tensor_max(rmax[:ibs, k0:k0 + kbs],
                                             rmax[:ibs, k0:k0 + kbs], asb[:ibs, :kbs])
                        ats = atsp.tile([128, 512], BF16, tag="ats", name=f"ats{li}_{kb}_{ib}")
                        nc.scalar.copy(ats[:ibs, :kbs], asb[:ibs, :kbs])
                        nc.sync.dma_start(atd[i0:i0 + ibs, k0:k0 + kbs], ats[:ibs, :kbs])

                def emit_cmax_ar(li):
                    for kc, (k0, kcs) in enumerate(KCH):
                        tp = tpp.tile([128, 128], F32, tag="tp", name=f"tpr{li}_{kc}")
                        nc.tensor.transpose(tp[:kcs, :], rmax[:, k0:k0 + kcs], ident[:, :])
                        nc.vector.reduce_max(cmax[:kcs, kc:kc + 1], tp[:kcs, :],
                                             axis=mybir.AxisListType.X)
                    nc.sync.dma_start(cm_in[:], cmax[:])
                    nc.gpsimd.collective_compute(
                        "AllReduce", ALU.max, replica_groups=RG,
                        ins=[cm_in[:].opt()], outs=[cm_out[:].opt()])

                def emit_hew(li):
                    nm = layers[li][0]
                    hw = hewp.tile([128, len(KCH), FE], BF16, tag="hew", name=f"hew{li}")
                    hew_t[li] = hw
                    for kc, (k0, kcs) in enumerate(KCH):
                        ps = hwp.tile([128, FE], F32, tag="pshew", name=f"ph{li}_{kc}")
                        nc.tensor.matmul(ps[:kcs, :], zht[:, k0:k0 + kcs], wl_sb[nm][:],
                                         start=True, stop=True)
                        nc.scalar.copy(hw[:kcs, kc, :], ps[:kcs, :])

                def emit_scale_hew(li):
                    hw = hew_t[li]
                    nc.sync.dma_start(cmax[:], cm_out[:])
                    nc.vector.reciprocal(invc[:], cmax[:])
                    for kc, (k0, kcs) in enumerate(KCH):
                        nc.vector.tensor_scalar_mul(hw[:kcs, kc, :], hw[:kcs, kc, :],
                                                    invc[:kcs, kc:kc + 1])

                def emit_pass2(li, last):
                    nm, _, _, bd = layers[li]
                    atd = at_dram[li % 2]
                    hw = hew_t[li]
                    pes = [enp.tile([FE, 384], F32, tag=f"pe{fb}", name=f"pe{li}_{fb}")
                           for fb in range(2)]
                    for kc, (k0, kcs) in enumerate(KCH):
                        att = auxp.tile([128, 768], BF16, tag="attb",
                                        name=f"att{li}_{kc}")
                        eng = nc.sync if kc % 2 == 0 else nc.scalar
                        eng.dma_start_transpose(
                            att[:, :], atd[0:768, kc * 128:(kc + 1) * 128])
                        for fb, f0 in enumerate((0, 384)):
                            nc.tensor.matmul(pes[fb][:, :], hw[:kcs, kc, :],
                                             att[:kcs, f0:f0 + 384],
                                             start=(kc == 0), stop=(kc == len(KCH) - 1))
                    for fb, f0 in enumerate((0, 384)):
                        fvalid = 384 if fb == 0 else MR - 384
                        nc.scalar.activation(enrelu[:, f0:f0 + fvalid],
                                             pes[fb][:, :fvalid],
                                             AF.Relu, bias=bl_sb[nm][:])
                    if not last:
                        nc.sync.dma_start(zg_in[:], enrelu[:])
                        nc.gpsimd.collective_compute(
                            "AllGather", ALU.bypass, replica_groups=RG,
                            ins=[zg_in[:].opt()], outs=[zg_out[:].opt()])
                        for r in range(CORES):
                            nc.sync.dma_start(zht[:, r * MR:(r + 1) * MR], zg_out[r, :, :])

                # pipelined emission: next layer's pass1 covers this layer's colmax AR
                emit_tsv(0)
                for kb in range(6):
                    emit_pass1_kb(0, kb)
                emit_g_and_u()
                for kb in range(6, len(KBL)):
                    emit_pass1_kb(0, kb)
                emit_cmax_ar(0)
                emit_hew(0)
                emit_tsv(1)
                for kb in range(3):
                    emit_pass1_kb(1, kb)
                emit_scale_hew(0)
                emit_pass2(0, last=False)
                for kb in range(3, len(KBL)):
                    emit_pass1_kb(1, kb)
                emit_cmax_ar(1)
                emit_hew(1)
                emit_tsv(2)
                for kb in range(3):
                    emit_pass1_kb(2, kb)
                emit_scale_hew(1)
                emit_pass2(1, last=False)
                for kb in range(3, len(KBL)):
                    emit_pass1_kb(2, kb)
                emit_cmax_ar(2)
                emit_hew(2)
                emit_scale_hew(2)
                emit_pass2(2, last=True)

            # ================= gc4 =================
            with (
                tc.tile_pool(name="g4", bufs=1) as g4,
                tc.tile_pool(name="g4s", bufs=3) as g4s,
                tc.tile_pool(name="g4p", bufs=2, space="PSUM") as g4p,
                tc.tile_pool(name="g4p1", bufs=1, space="PSUM") as g4p1,
            ):
                # se from the core's own relu'd Zh piece
                for kb, (k0, ksz) in enumerate(IBL):
                    ps = g4p1.tile([128, 1], F32, tag="small")
                    nc.tensor.matmul(ps[:ksz, :], enrelu[:, k0:k0 + ksz], pv4[:],
                                     start=True, stop=True)
                    nc.scalar.copy(secol[:ksz, kb:kb + 1], ps[:ksz, :])
                # scale precomputed U by se
                u_sb = g4.tile([128, len(IBL), H2], BF16)
                for kb, (k0, ksz) in enumerate(IBL):
                    nc.vector.tensor_scalar_mul(u_sb[:ksz, kb, :], u_raw[:ksz, kb, :],
                                                secol[:ksz, kb:kb + 1])
                # VT partial = (U*se).T-accum over local edge rows x TT slab
                for vb, (v0, vsz) in enumerate(VFB):
                    ps = g4p.tile([H2, 512], F32, tag="psvt")
                    for kb, (k0, ksz) in enumerate(IBL):
                        ttt = g4s.tile([128, 512], BF16, tag="ttt")
                        nc.sync.dma_start(ttt[:ksz, :vsz], ttslab_d[k0:k0 + ksz, v0:v0 + vsz])
                        nc.tensor.matmul(ps[:, :vsz], (u_sb[:ksz, kb, :]),
                                         (ttt[:ksz, :vsz]),
                                         start=(kb == 0), stop=(kb == len(IBL) - 1))
                    vst = g4s.tile([H2, 512], BF16, tag="vst")
                    nc.scalar.copy(vst[:, :vsz], ps[:, :vsz])
                    nc.sync.dma_start(vt_in[:, v0:v0 + vsz], vst[:, :vsz])
                nc.gpsimd.collective_compute(
                    "AllReduce", ALU.add, replica_groups=RG,
                    ins=[vt_in[:].opt()], outs=[vt_out[:].opt()])

            # ================= gc5 + softmax =================
            with (
                tc.tile_pool(name="g5", bufs=1) as g5,
                tc.tile_pool(name="g5s", bufs=3) as g5s,
                tc.tile_pool(name="g5p", bufs=2, space="PSUM") as g5p,
                tc.tile_pool(name="g5pt", bufs=1, space="PSUM") as g5pt,
            ):
                xh5t = g5.tile([H2, N], BF16)
                nc.sync.dma_start(xh5t[:], vt_out[:])
                nc.scalar.activation(xh5t[:], xh5t[:], AF.Relu, bias=b4[:])
                xw5 = g5.tile([128, len(NCH), NCLS], BF16)
                for ci, (ns, nsz) in enumerate(NCH):
                    ps = g5p.tile([128, NCLS], F32, tag="psw5")
                    nc.tensor.matmul(ps[:nsz, :], xh5t[:, ns:ns + nsz], w5[:],
                                     start=True, stop=True)
                    nc.scalar.copy(xw5[:nsz, ci, :], ps[:nsz, :])
                pst = g5pt.tile([NCLS, 375], F32)
                for ci, (ns, nsz) in enumerate(NCH):
                    ptc = g5s.tile([128, NR], BF16, tag="ptc5")
                    nc.sync.dma_start(ptc[:nsz, :], ptcol_d[ns:ns + nsz, :])
                    nc.tensor.matmul(pst[:, :], (xw5[:nsz, ci, :]), (ptc[:nsz, :]),
                                     start=(ci == 0), stop=(ci == len(NCH) - 1))
                st_sb = g5.tile([NCLS, NR], F32)
                nc.vector.tensor_scalar_add(st_sb[:], pst[:, :], b5[:])
                outt = g5.tile([128, len(NLB), NCLS], F32)
                ptp = g5pt.tile([128, len(NLB), NCLS], F32)
                for j, (t0, tsz) in enumerate(NLB):
                    nc.tensor.transpose(ptp[:tsz, j, :], st_sb[:, t0:t0 + tsz],
                                        ident[:NCLS, :NCLS])
                    red = g5s.tile([128, 1], F32, tag="red5")
                    nc.vector.reduce_max(red[:tsz, :], ptp[:tsz, j, :],
                                         axis=mybir.AxisListType.X)
                    nc.vector.tensor_scalar_mul(red[:tsz, :], red[:tsz, :], -1.0)
                    nc.scalar.activation(outt[:tsz, j, :], ptp[:tsz, j, :], AF.Exp,
                                         bias=red[:tsz, :])
                    ssum = g5s.tile([128, 1], F32, tag="ssum5")
                    nc.vector.reduce_sum(ssum[:tsz, :], outt[:tsz, j, :],
                                         axis=mybir.AxisListType.X)
                    nc.vector.reciprocal(ssum[:tsz, :], ssum[:tsz, :])
                    nc.vector.tensor_scalar_mul(outt[:tsz, j, :], outt[:tsz, j, :],
                                                ssum[:tsz, :])
                    nc.sync.dma_start(out_d[t0:t0 + tsz, :], outt[:tsz, j, :])

    nc.finalize()
    return nc


def prepare_inputs(inputs):
    f = lambda x: np.ascontiguousarray(np.asarray(x), dtype=np.float32)
    X, Z, adj_e, T = f(inputs["X"]), f(inputs["Z"]), f(inputs["adj_e"]), f(inputs["T"])
    ei = np.asarray(inputs["edge_index"])
    W1, b1 = f(inputs["W1"]), f(inputs["b1"])
    p2, W2, b2 = f(inputs["p2"]), f(inputs["W2"]), f(inputs["b2"])
    p3, W3, b3 = f(inputs["p3"]), f(inputs["W3"]), f(inputs["b3"])
    p32, W32, b32 = f(inputs["p32"]), f(inputs["W32"]), f(inputs["b32"])
    p4, W4 = f(inputs["p4"]), f(inputs["W4"])
    b4g, b4, W5, b5 = f(inputs["b4g"]), f(inputs["b4"]), f(inputs["W5"]), f(inputs["b5"])

    # dense PT = P.T where P is the symmetric-normalized (A+I) propagation matrix
    src = ei[0].astype(np.int64)
    dst = ei[1].astype(np.int64)
    loop = np.arange(N, dtype=np.int64)
    s = np.concatenate([src, loop])
    d = np.concatenate([dst, loop])
    deg = np.zeros(N, np.float32)
    np.add.at(deg, d, np.float32(1.0))
    dinv = np.where(deg > 0, 1.0 / np.sqrt(deg), 0.0).astype(np.float32)
    norm = dinv[s] * dinv[d]
    PT = np.zeros((N, N), np.float32)
    np.add.at(PT, (s, d), norm)

    import ml_dtypes
    bf16 = ml_dtypes.bfloat16
    fp8 = ml_dtypes.float8_e4m3
    Tpad = np.zeros((3072, M), np.float32)
    Tpad[:N] = T
    kiota = np.tile(np.arange(512, dtype=np.float32), (128, 1))
    base = dict(
        tpad=np.ascontiguousarray(Tpad.astype(fp8)),
        xt=np.ascontiguousarray(X.T.astype(bf16)), zt=np.ascontiguousarray(Z.T),
        kiota=kiota, w1=np.ascontiguousarray(W1.astype(bf16)), w2=W2, w3=W3, w32=W32,
        w4=np.ascontiguousarray(np.transpose(W4.reshape(2, 128, H2), (1, 0, 2))),
        w5=np.ascontiguousarray(W5.astype(bf16)),
        pv2=np.ascontiguousarray(p2[0].reshape(2, 128).T),
        pv3=np.ascontiguousarray(p3[0].reshape(2, 128).T),
        pv32=np.ascontiguousarray(p32[0].reshape(2, 128).T),
        pv4=np.ascontiguousarray(p4[0][:, None]),
        b1=np.ascontiguousarray(b1.reshape(2, 128).T),
        b2=b2[:, None], b3=b3[:, None], b32=b32[:, None],
        b4g=b4g[None, :], b4=b4[:, None], b5=b5[:, None],
    )
    in_maps = []
    for c in range(CORES):
        e0, n0 = c * MR, c * NR
        m = dict(base)
        m["tslab"] = np.ascontiguousarray(Tpad[:, e0:e0 + MR].astype(bf16))
        m["ttslab"] = np.ascontiguousarray(T[:, e0:e0 + MR].T.astype(bf16))
        m["adjslab"] = np.ascontiguousarray(adj_e[e0:e0 + MR, :])
        m["ptcol"] = np.ascontiguousarray(PT[:, n0:n0 + NR].astype(bf16))
        m["ptrow"] = np.ascontiguousarray(PT[n0:n0 + NR, :])
        m["cidx"] = _col_layout(np.arange(e0, e0 + MR, dtype=np.float32), 128)
        m["cidx"][110:, 5] = -1.0  # pad slots beyond row 750 must never match
        m["vfix"] = _col_layout(np.diagonal(adj_e[e0:e0 + MR, e0:e0 + MR]).astype(np.float32), 128)
        in_maps.append({k: (np.ascontiguousarray(v) if v.dtype in (bf16, fp8)
                            else np.ascontiguousarray(v, dtype=np.float32))
                        for k, v in m.items()})
    return in_maps


_CACHE = {}


def kernel(**inputs):
    in_maps = prepare_inputs(inputs)
    if "nc" not in _CACHE:
        _CACHE["nc"] = build_program()
    res = bass_utils.run_bass_kernel_spmd(_CACHE["nc"], in_maps, list(range(CORES)))
    out = np.concatenate([res.results[c]["out"] for c in range(CORES)], axis=0)
    return out.astype(np.float32)


if __name__ == "__main__":
    import reference
    ins = reference.setup_inputs()
    ins = {k: np.asarray(v) for k, v in ins.items()}
    got = kernel(**ins)
    print("kernel output", got.shape, got.dtype)
